# revision 10
# baseline (speedup 1.0000x reference)
"""Trainium2 Bass kernel for BipartiteGCN (8 NeuronCores, SPMD). v2

Strategy (v2 — scheduling-first rewrite):
 - Node rows sharded 8 ways; edges sharded by DESTINATION range, sorted by
   dst block then src; per-edge lp[src] rows fetched via SWDGE dma_gather.
 - SWDGE descriptor generation (~7.7ns/row, ~2.0ms/core total) is the hard
   floor; everything else must overlap it:
     * gathers issued as prepare_only descriptor preps with a W-deep
       sliding window flattened across BOTH convs; triggers fire when the
       lp tables land. GPSIMD starts prepping at t=0 (during embeds) and
       never idles.
     * pools are deep enough that DVE stats of chunk k never WAR-wait on
       gather k-4 completions (the v1 failure mode: 1.7ms of DVE stalls).
 - Scatter-mean division eliminated exactly: LN is scale-invariant per row,
   so LN(agg/cnt + bf) == LN(agg + cnt*bf) with eps -> eps*max(cnt,1)^2
   (cnt is host-known index data, shipped as per-partition constants).
 - All rsqrt on DVE via Newton iteration (no ACT Sqrt table churn).
 - Embed / conv-post / heads as v1 otherwise.
"""

import os
import sys

for _p in ("/opt/trn_rl_repo",):
    if _p not in sys.path:
        sys.path.insert(0, _p)

import numpy as np
import ml_dtypes

import concourse.bass as bass
import concourse.bacc as bacc
import concourse.mybir as mybir
from concourse import tile, library_config
from concourse.bass_utils import run_bass_kernel_spmd
from concourse import hw_specs as _hw_specs

# Calibrated SWDGE dma_gather descriptor-generation rate (measured ~7.7
# ns/descriptor on HW; the stock 0.34 makes the Tile scheduler mis-order).
_hw_specs.TRN2Spec.SWDGE_NS_PER_DESCRIPTOR = 7.7

BF16 = ml_dtypes.bfloat16
F32 = np.float32
NCORES = 8
EMB = 128
CHUNK_TILES = 20     # tiles (128 edges) per dma_gather
PREP_AHEAD = 4       # gather chunks prepped ahead of their trigger
PREP_MODE = os.environ.get("KPREP", "prep")  # "prep" | "inline"
HI_BASE = 32768
EPS = 1e-5
SLOPE = 0.01

dt = mybir.dt


def _wrap_idx(idx_i16):
    """[N] int16 -> [128, N//16] wrapped (i at [i%16, i//16]) + replicated 8x."""
    n = idx_i16.shape[0]
    assert n % 16 == 0
    w = idx_i16.reshape(n // 16, 16).T
    return np.tile(w, (8, 1)).copy()


def _bcast_row(v, rows=128):
    return np.broadcast_to(np.asarray(v, F32)[None, :], (rows, v.shape[0])).copy()


class ConvPrep:
    """Per-conv edge-sharding data. Same segment layout for all cores."""

    def __init__(self, dst, src, n_dst, n_src, dst_per_core):
        self.n_dst_local = dst_per_core
        self.nblocks = -(-dst_per_core // 128)
        nb = self.nblocks
        self.two_buckets = n_src > HI_BASE
        nu = 2 if self.two_buckets else 1
        self.nu = nu

        core = dst // dst_per_core
        dloc_all = dst - core * dst_per_core
        block_all = dloc_all // 128

        per = [[[None] * nb for _ in range(nu)] for _ in range(NCORES)]
        for c in range(NCORES):
            m = core == c
            d_c = dloc_all[m]
            s_c = src[m]
            b_c = block_all[m]
            u_c = (s_c >= HI_BASE).astype(np.int8) if self.two_buckets else np.zeros(
                len(s_c), np.int8
            )
            for u in range(nu):
                mu = u_c == u
                db, sb, bb = d_c[mu], s_c[mu], b_c[mu]
                order = np.argsort(bb, kind="stable")
                db, sb, bb = db[order], sb[order], bb[order]
                bounds = np.searchsorted(bb, np.arange(nb + 1))
                for b in range(nb):
                    lo, hi = bounds[b], bounds[b + 1]
                    o2 = np.argsort(sb[lo:hi], kind="stable")
                    per[c][u][b] = (sb[lo:hi][o2], db[lo:hi][o2])

        self.ntiles = np.zeros((nu, nb), np.int64)
        for u in range(nu):
            for b in range(nb):
                mx = max(len(per[c][u][b][0]) for c in range(NCORES))
                self.ntiles[u, b] = -(-mx // 128) if mx > 0 else 0

        etot = int(self.ntiles.sum()) * 128
        self.etot = etot

        # host-known scatter-mean counts: cnt and EPS*max(cnt,1)^2
        self.cnt = np.zeros((NCORES, 128, nb), F32)
        self.eps2 = np.zeros((NCORES, 128, nb), F32)
        for c in range(NCORES):
            cnt = np.bincount(dloc_all[core == c], minlength=nb * 128).astype(F32)
            cnt = cnt[: nb * 128].reshape(nb, 128).T  # [128, nb]
            self.cnt[c] = cnt
            self.eps2[c] = EPS * np.maximum(cnt, 1.0) ** 2

        self.src_idx = np.zeros((NCORES, etot), np.int16)
        self.dstrel = np.full((NCORES, etot), -1.0, F32)
        off = 0
        self.seg_offsets = {}
        for u in range(nu):
            for b in range(nb):
                g = int(self.ntiles[u, b])
                if g == 0:
                    continue
                self.seg_offsets[(u, b)] = off
                for c in range(NCORES):
                    sb, db = per[c][u][b]
                    n = len(sb)
                    s_adj = sb - (HI_BASE if u == 1 else 0)
                    self.src_idx[c, off : off + n] = s_adj.astype(np.int16)
                    self.dstrel[c, off : off + n] = (db - 128 * b).astype(F32)
                off += g * 128
        assert off == etot

        self.streams = []
        for u in range(nu):
            blocks = [(b, int(self.ntiles[u, b])) for b in range(nb) if self.ntiles[u, b] > 0]
            start = self.seg_offsets[(u, blocks[0][0])] if blocks else 0
            nt = sum(g for _, g in blocks)
            self.streams.append({"u": u, "blocks": blocks, "start_edge": start, "ntiles": nt})

    def core_arrays(self, c):
        dr = self.dstrel[c]
        i = np.nonzero(dr >= 0)[0]
        lane = i % 128
        tb = (i // 128) * 128
        d = dr[i].astype(np.int64)
        oh = np.zeros((128, self.etot), ml_dtypes.float8_e4m3)
        oh[lane, tb + d] = 1.0
        ohT = np.zeros((128, self.etot), ml_dtypes.float8_e4m3)
        ohT[d, tb + lane] = 1.0
        return _wrap_idx(self.src_idx[c]), oh, ohT


def host_prep(inputs):
    p = {}
    cons_x = np.asarray(inputs["cons_x"], F32)
    var_x = np.asarray(inputs["var_x"], F32)
    edge_cons = np.asarray(inputs["edge_cons"]).astype(np.int64)
    edge_var = np.asarray(inputs["edge_var"]).astype(np.int64)
    head_mask = np.asarray(inputs["head_mask"]).astype(bool)

    NC, CF = cons_x.shape
    NV, VF = var_x.shape
    assert NC % NCORES == 0 and NV % NCORES == 0
    NCL, NVL = NC // NCORES, NV // NCORES
    p.update(NC=NC, NV=NV, CF=CF, VF=VF, NCL=NCL, NVL=NVL)

    p["conv1"] = ConvPrep(edge_cons, edge_var, NC, NV, NCL)
    p["conv2"] = ConvPrep(edge_var, edge_cons, NV, NC, NVL)

    w = {}

    def embed_w(prefix, g, b, w1, b1, w2, b2, feat):
        w1 = np.asarray(w1, F32)
        w1g = np.asarray(g, F32)[:, None] * w1
        aug = np.concatenate([w1g, np.zeros((1, w1.shape[1]), F32)], 0)
        w[prefix + "w1aug"] = aug.astype(BF16)
        w[prefix + "s1"] = _bcast_row(np.asarray(b, F32) @ w1 + np.asarray(b1, F32))
        w[prefix + "r1"] = _bcast_row(w1g.sum(0))
        w[prefix + "w2"] = np.asarray(w2, F32).astype(BF16)
        w[prefix + "b2"] = _bcast_row(np.asarray(b2, F32))

    embed_w("ce_", inputs["ce_ln_g"], inputs["ce_ln_b"], inputs["ce_w1"],
            inputs["ce_b1"], inputs["ce_w2"], inputs["ce_b2"], CF)
    embed_w("ve_", inputs["ve_ln_g"], inputs["ve_ln_b"], inputs["ve_w1"],
            inputs["ve_b1"], inputs["ve_w2"], inputs["ve_b2"], VF)

    for pre in ("vc_", "cv_"):
        wl = np.asarray(inputs[pre + "wl"], F32)
        w[pre + "wl"] = wl.astype(BF16)
        w[pre + "bl"] = _bcast_row(np.asarray(inputs[pre + "bl"], F32))
        w[pre + "wr"] = np.asarray(inputs[pre + "wr"], F32).astype(BF16)
        flg = np.asarray(inputs[pre + "flg"], F32)
        flb = np.asarray(inputs[pre + "flb"], F32)
        p[pre + "fl_trivial"] = bool(np.all(flg == 1.0) and np.all(flb == 0.0))
        w[pre + "flg"] = _bcast_row(flg)
        w[pre + "flb"] = _bcast_row(flb)
        w[pre + "wf"] = np.asarray(inputs[pre + "wf"], F32).astype(BF16)
        w[pre + "bf"] = _bcast_row(np.asarray(inputs[pre + "bf"], F32))
        wo1 = np.asarray(inputs[pre + "wo1"], F32)
        plg = np.asarray(inputs[pre + "plg"], F32)
        plb = np.asarray(inputs[pre + "plb"], F32)
        w[pre + "wo1a"] = (plg[:, None] * wo1[:EMB]).astype(BF16)
        w[pre + "wo1b"] = wo1[EMB:].astype(BF16)
        w[pre + "bo1"] = _bcast_row(np.asarray(inputs[pre + "bo1"], F32) + plb @ wo1[:EMB])
        w[pre + "wo2"] = np.asarray(inputs[pre + "wo2"], F32).astype(BF16)
        w[pre + "bo2"] = _bcast_row(np.asarray(inputs[pre + "bo2"], F32))

    active = np.nonzero(head_mask)[0]
    nact = int(len(active))
    p["nact"] = nact
    denom = max(float(head_mask.sum()), 1.0)
    hb2 = np.asarray(inputs["hb2"], F32)
    p["out_scale"] = 1.0 / denom
    p["out_add"] = float(hb2[active].sum() / denom)
    if nact > 0:
        hw1 = np.asarray(inputs["hw1"], F32)[active]
        w["hw1"] = hw1.transpose(1, 0, 2).astype(BF16).copy()
        w["hb1"] = np.asarray(inputs["hb1"], F32)[active].T.copy()
        w["hw2"] = np.asarray(inputs["hw2"], F32)[active].T.astype(BF16).copy()

    w["identity"] = np.eye(128, dtype=BF16)
    p["weights"] = w

    NCLp = -(-NCL // 128) * 128
    NVLp = -(-NVL // 128) * 128
    p.update(NCLp=NCLp, NVLp=NVLp)
    NVLh = -(-NVL // 512) * 512
    p["NVLh"] = NVLh

    core_inputs = []
    for c in range(NCORES):
        m = {}
        cx = cons_x[c * NCL : (c + 1) * NCL]
        vx = var_x[c * NVL : (c + 1) * NVL]
        cxp = np.zeros((NCLp, CF), F32)
        cxp[:NCL] = cx
        vxp = np.zeros((NVLp, VF), F32)
        vxp[:NVL] = vx
        m["cons_rows"] = cxp.reshape(NCLp // 128, 128, CF).transpose(1, 0, 2).copy()
        m["var_rows"] = vxp.reshape(NVLp // 128, 128, VF).transpose(1, 0, 2).copy()
        m["consT_aug"] = np.concatenate([cxp.T, np.ones((1, NCLp), F32)], 0).astype(BF16)
        m["varT_aug"] = np.concatenate([vxp.T, np.ones((1, NVLp), F32)], 0).astype(BF16)
        s1, oh1, ohT1 = p["conv1"].core_arrays(c)
        m["e1_src"], m["e1_oh"], m["e1_ohT"] = s1, oh1, ohT1
        s2, oh2, ohT2 = p["conv2"].core_arrays(c)
        m["e2_src"], m["e2_oh"], m["e2_ohT"] = s2, oh2, ohT2
        m["e1_cnt"] = p["conv1"].cnt[c]
        m["e1_eps"] = p["conv1"].eps2[c]
        m["e2_cnt"] = p["conv2"].cnt[c]
        m["e2_eps"] = p["conv2"].eps2[c]
        for k, v in w.items():
            m[k] = v
        core_inputs.append(m)
    p["core_inputs"] = core_inputs
    return p


# ---------------------------------------------------------------------------


class B:
    def __init__(self, p):
        self.p = p
        self.nc = bacc.Bacc("TRN2", target_bir_lowering=False, debug=False,
                            num_devices=NCORES, num_swdge_queues=2)
        self.d = {}

    def dram(self, name, shape, dtype, kind=None, addr_space=None):
        kw = {}
        if kind:
            kw["kind"] = kind
        if addr_space:
            kw["addr_space"] = addr_space
        t = self.nc.dram_tensor(name, list(shape), dtype, **kw)
        self.d[name] = t
        return t


MAGIC = 0x5F3759DF


def rsqrt_newton(nc, pool, src_ap, n, tag, iters=2):
    """1/sqrt(src) on DVE only. src_ap [128, n] f32 > 0."""
    AL = mybir.AluOpType
    sh = pool.tile([128, n], dt.int32, tag=tag + "sh")
    nc.vector.tensor_scalar(sh[:], src_ap.bitcast(dt.int32), 1, None,
                            AL.arith_shift_right)
    y0 = pool.tile([128, n], dt.int32, tag=tag + "y0")
    nc.vector.tensor_scalar(y0[:], sh[:], -1, MAGIC, AL.mult, AL.add)
    cur = y0[:].bitcast(dt.float32)
    h = pool.tile([128, n], dt.float32, tag=tag + "h")
    nc.vector.tensor_scalar_mul(h[:], src_ap, 0.5)
    for it in range(iters):
        yy = pool.tile([128, n], dt.float32, tag=tag + f"yy{it}")
        nc.vector.tensor_tensor(yy[:], cur, cur, AL.mult)
        nc.vector.tensor_tensor(yy[:], yy[:], h[:], AL.mult)
        nc.vector.tensor_scalar(yy[:], yy[:], -1.0, 1.5, AL.mult, AL.add)
        nxt = pool.tile([128, n], dt.float32, tag=tag + f"n{it}")
        nc.vector.tensor_tensor(nxt[:], cur, yy[:], AL.mult)
        cur = nxt[:]
    return cur


def build_program(p):
    b = B(p)
    nc = b.nc
    w = p["weights"]
    NCL, NVL, NCLp, NVLp = p["NCL"], p["NVL"], p["NCLp"], p["NVLp"]
    CF, VF = p["CF"], p["VF"]
    NC, NV = p["NC"], p["NV"]
    NVLh = p["NVLh"]
    nact = p["nact"]

    din = lambda n, s, t: b.dram(n, s, t, kind="ExternalInput")
    din("cons_rows", [128, NCLp // 128, CF], dt.float32)
    din("var_rows", [128, NVLp // 128, VF], dt.float32)
    din("consT_aug", [CF + 1, NCLp], dt.bfloat16)
    din("varT_aug", [VF + 1, NVLp], dt.bfloat16)
    c1p, c2p = p["conv1"], p["conv2"]
    din("e1_src", [128, c1p.etot // 16], dt.int16)
    din("e1_oh", [128, c1p.etot], dt.float8e4)
    din("e1_ohT", [128, c1p.etot], dt.float8e4)
    din("e2_src", [128, c2p.etot // 16], dt.int16)
    din("e2_oh", [128, c2p.etot], dt.float8e4)
    din("e2_ohT", [128, c2p.etot], dt.float8e4)
    din("e1_cnt", [128, c1p.nblocks], dt.float32)
    din("e1_eps", [128, c1p.nblocks], dt.float32)
    din("e2_cnt", [128, c2p.nblocks], dt.float32)
    din("e2_eps", [128, c2p.nblocks], dt.float32)
    for k, v in w.items():
        dtt = dt.bfloat16 if v.dtype == BF16 else (dt.int16 if v.dtype == np.int16 else dt.float32)
        din(k, list(v.shape), dtt)
    out_d = b.dram("out", [1, NVLh], dt.bfloat16, kind="ExternalOutput")

    lp1_loc = b.dram("lp1_loc", [NVL, EMB], dt.bfloat16)
    lp1_full = b.dram("lp1_full", [NV, EMB], dt.bfloat16, addr_space="Shared")
    rp1_loc = b.dram("rp1_loc", [NCL, EMB], dt.bfloat16)
    lp2_loc = b.dram("lp2_loc", [NCL, EMB], dt.bfloat16)
    lp2_full = b.dram("lp2_full", [NC, EMB], dt.bfloat16, addr_space="Shared")
    rp2_loc = b.dram("rp2_loc", [NVL, EMB], dt.bfloat16)

    LR = mybir.ActivationFunctionType.Lrelu
    AL = mybir.AluOpType

    with tile.TileContext(nc) as tc:
        nc.gpsimd.load_library(library_config.mlp)
        with (
            tc.tile_pool(name="const", bufs=1) as cpool,
            tc.tile_pool(name="resident", bufs=1) as rpool,
            tc.tile_pool(name="work", bufs=3) as wpool,
            tc.tile_pool(name="tiny", bufs=4) as tpool,
            tc.tile_pool(name="gath", bufs=PREP_AHEAD + 2) as gpool,
            tc.tile_pool(name="sidxp", bufs=PREP_AHEAD + 2) as sidxpool,
            tc.tile_pool(name="ohp", bufs=2) as ohpool,
            tc.tile_pool(name="xwp", bufs=2) as xwpool,
            tc.tile_pool(name="embp", bufs=1) as embp,
            tc.tile_pool(name="postp", bufs=2) as postp,
            tc.tile_pool(name="headp", bufs=2) as headp,
            tc.tile_pool(name="psA", bufs=3, space="PSUM") as psA,
            tc.tile_pool(name="psT", bufs=1, space="PSUM") as psT,
            tc.tile_pool(name="psagg", bufs=2, space="PSUM") as psagg,
            tc.tile_pool(name="psout", bufs=1, space="PSUM") as psout,
        ):
            # ---- constants ----
            cw = {}
            for k, v in w.items():
                if k == "hw1":
                    continue  # streamed during the head stage
                dtt = dt.bfloat16 if v.dtype == BF16 else dt.float32
                t = cpool.tile(list(v.shape), dtt, tag=k)
                nc.sync.dma_start(t[:], b.d[k][:])
                cw[k] = t

            ident = cw["identity"]
            zero_col = cpool.tile([128, 1], dt.float32, tag="zero_col")
            nc.vector.memset(zero_col[:], 0.0)

            cnt1 = cpool.tile([128, c1p.nblocks], dt.float32, tag="cnt1")
            nc.sync.dma_start(cnt1[:], b.d["e1_cnt"][:])
            eps1 = cpool.tile([128, c1p.nblocks], dt.float32, tag="eps1")
            nc.sync.dma_start(eps1[:], b.d["e1_eps"][:])
            cnt2 = cpool.tile([128, c2p.nblocks], dt.float32, tag="cnt2")
            nc.sync.dma_start(cnt2[:], b.d["e2_cnt"][:])
            eps2 = cpool.tile([128, c2p.nblocks], dt.float32, tag="eps2")
            nc.sync.dma_start(eps2[:], b.d["e2_eps"][:])

            # residents
            c0T = rpool.tile([128, NCLp], dt.bfloat16, tag="c0T")
            v0T = rpool.tile([128, NVLp], dt.bfloat16, tag="v0T")
            c1T = rpool.tile([128, NCLp], dt.bfloat16, tag="c1T")
            v1T = rpool.tile([128, NVLh], dt.bfloat16, tag="v1T")
            nc.vector.memset(v1T[:], 0.0)
            acc1 = rpool.tile([128, c1p.nblocks, EMB], dt.bfloat16, tag="acc1")
            nc.vector.memset(acc1[:], 0.0)
            acc2 = rpool.tile([128, c2p.nblocks, EMB], dt.bfloat16, tag="acc2")
            nc.vector.memset(acc2[:], 0.0)

            # ---- SWDGE prep/trigger machinery ----
            dma_sems = [nc.alloc_semaphore("gq0"), nc.alloc_semaphore("gq1")]
            prep_sem = nc.alloc_semaphore("prep_ctr")
            gp_last = [None]

            def gp_chain(inst):
                if gp_last[0] is not None:
                    deps = bass.InstructionNameOrderedSet()
                    deps.add(gp_last[0])
                    inst.ins.add_nosync_dependencies_from(deps)
                gp_last[0] = inst.ins.name
                return inst

            # flattened chunk list across conv1 then conv2
            chunks = []
            for conv_id, cv, lp_dram, src_d in (
                (1, c1p, lp1_full, b.d["e1_src"]),
                (2, c2p, lp2_full, b.d["e2_src"]),
            ):
                for stream in cv.streams:
                    base_edge = stream["start_edge"]
                    ntiles = stream["ntiles"]
                    view_lo = HI_BASE if stream["u"] == 1 else 0
                    blk_of_tile = {}
                    t0 = 0
                    for (blk, tcnt) in stream["blocks"]:
                        for t in range(t0, t0 + tcnt):
                            blk_of_tile[t] = (blk, t == t0, t == t0 + tcnt - 1)
                        t0 += tcnt
                    tdone = 0
                    while tdone < ntiles:
                        tcn = min(CHUNK_TILES, ntiles - tdone)
                        chunks.append(dict(
                            conv=conv_id, lp=lp_dram, src_d=src_d,
                            view_lo=view_lo, e0=base_edge + tdone * 128,
                            t0=tdone, tcn=tcn, blk_of_tile=blk_of_tile,
                        ))
                        tdone += tcn
            for gi, ch in enumerate(chunks):
                ch["q"] = gi % 2

            prep_count = [0]

            def emit_prep(ch):
                ne = ch["tcn"] * 128
                e0 = ch["e0"]
                sidx = sidxpool.tile([128, CHUNK_TILES * 8], dt.int16, tag="sidx")
                nc.sync.dma_start(sidx[:, : ne // 16],
                                  ch["src_d"][:, e0 // 16 : (e0 + ne) // 16])
                g = gpool.tile([128, CHUNK_TILES, EMB], dt.bfloat16, tag="sgat")
                ch["g"] = g
                ch["sidx"] = sidx
                if PREP_MODE != "prep":
                    return
                lp_view = ch["lp"][ch["view_lo"]:, :] if ch["view_lo"] else ch["lp"][:, :]
                inst = nc.gpsimd.dma_gather(
                    g[:, : ch["tcn"], :], lp_view, sidx[:, : ne // 16], ne, ne,
                    EMB, single_packet=False, prepare_only=True,
                    sem=dma_sems[ch["q"]], queue_num=ch["q"])
                inst.then_inc(prep_sem, 1)
                gp_chain(inst)
                prep_count[0] += 1
                ch["prep_no"] = prep_count[0]

            def emit_trigger(ch):
                if PREP_MODE != "prep":
                    ne = ch["tcn"] * 128
                    lp_view = (ch["lp"][ch["view_lo"]:, :] if ch["view_lo"]
                               else ch["lp"][:, :])
                    gp_chain(nc.gpsimd.dma_gather(
                        ch["g"][:, : ch["tcn"], :], lp_view,
                        ch["sidx"][:, : ne // 16], ne, ne, EMB,
                        single_packet=False, queue_num=ch["q"]))
                    return
                gp_chain(nc.gpsimd.wait_ge(prep_sem, ch["prep_no"]))
                gp_chain(nc.gpsimd.trigger_dma(count=1, queue_num=ch["q"]))

            # =========== embeddings ===========
            def transpose_to(dst_ap, src_ap, n_p, n_f):
                ps = psT.tile([128, 128], dt.bfloat16, tag="psT")
                nc.tensor.transpose(ps[:n_f, :n_p], src_ap, ident[:n_p, :n_p])
                nc.scalar.copy(dst_ap, ps[:n_f, :n_p])

            def embed(pre, xT_aug_name, rows_name, nrows_p, nfeat, outT, extra):
                nchunks = nrows_p // 128
                xall = embp.tile([128, nchunks, nfeat], dt.float32, tag="embx")
                nc.sync.dma_start(xall[:], b.d[rows_name][:])
                sx = tpool.tile([128, nchunks], dt.float32, tag="esx")
                nc.vector.reduce_sum(sx[:], xall[:], axis=mybir.AxisListType.X)
                nc.vector.tensor_tensor(xall[:], xall[:], xall[:], AL.mult)
                sxx = tpool.tile([128, nchunks], dt.float32, tag="esxx")
                nc.vector.reduce_sum(sxx[:], xall[:], axis=mybir.AxisListType.X)
                inv = 1.0 / nfeat
                mu_b = tpool.tile([128, nchunks], dt.float32, tag="emub")
                nc.vector.tensor_scalar_mul(mu_b[:], sx[:], inv)
                veps = tpool.tile([128, nchunks], dt.float32, tag="evep")
                nc.vector.tensor_scalar(veps[:], sxx[:], inv, EPS, AL.mult, AL.add)
                nmusq = tpool.tile([128, nchunks], dt.float32, tag="enmu")
                nc.vector.scalar_tensor_tensor(
                    nmusq[:], mu_b[:], -1.0, mu_b[:], AL.mult, AL.mult)
                nc.vector.tensor_tensor(veps[:], veps[:], nmusq[:], AL.add)
                rstd_b = rsqrt_newton(nc, tpool, veps[:], nchunks, "erst")
                nrstd_b = tpool.tile([128, nchunks], dt.float32, tag="enrs")
                nc.vector.tensor_scalar_mul(nrstd_b[:], rstd_b, -1.0)
                for chn in range(nchunks):
                    xTa = wpool.tile([nfeat + 1, 128], dt.bfloat16, tag="xTa")
                    nc.sync.dma_start(xTa[:], b.d[xT_aug_name][:, chn * 128 : (chn + 1) * 128])
                    ps = psA.tile([128, EMB], dt.float32, tag="ps")
                    nc.tensor.matmul(ps[:], xTa[:],
                                     cw[pre + "w1aug"][:], start=True, stop=True)
                    tmid = wpool.tile([128, EMB], dt.float32, tag="embmid")
                    nc.vector.scalar_tensor_tensor(
                        tmid[:], cw[pre + "r1"][:], mu_b[:, chn : chn + 1], ps[:],
                        AL.mult, AL.subtract)
                    tmid2 = wpool.tile([128, EMB], dt.float32, tag="tmid2")
                    nc.vector.scalar_tensor_tensor(
                        tmid2[:], tmid[:], nrstd_b[:, chn : chn + 1], cw[pre + "s1"][:],
                        AL.mult, AL.add)
                    z1 = wpool.tile([128, EMB], dt.bfloat16, tag="z1")
                    nc.scalar.activation(z1[:], tmid2[:], LR, bias=zero_col[:], alpha=SLOPE)
                    z1T = wpool.tile([128, 128], dt.bfloat16, tag="z1T")
                    transpose_to(z1T[:], z1[:], 128, 128)
                    ps2 = psA.tile([128, EMB], dt.float32, tag="ps")
                    nc.tensor.matmul(ps2[:], z1T[:], cw[pre + "w2"][:], start=True, stop=True)
                    u = wpool.tile([128, EMB], dt.float32, tag="embu")
                    nc.vector.tensor_add(u[:], ps2[:], cw[pre + "b2"][:])
                    z2 = wpool.tile([128, EMB], dt.bfloat16, tag="z2")
                    nc.scalar.activation(z2[:], u[:], LR, bias=zero_col[:], alpha=SLOPE)
                    transpose_to(outT[:, chn * 128 : (chn + 1) * 128], z2[:], 128, 128)
                    for (wname, bname, dout, n_valid, odt) in extra:
                        lo = chn * 128
                        nv = min(128, max(0, n_valid - lo))
                        if nv == 0:
                            continue
                        ps3 = psA.tile([128, EMB], dt.float32, tag="ps")
                        nc.tensor.matmul(ps3[:], outT[:, lo : lo + 128],
                                         cw[wname][:], start=True, stop=True)
                        ob = wpool.tile([128, EMB], odt, tag="projo")
                        if bname is not None:
                            ub = wpool.tile([128, EMB], dt.float32, tag="proju")
                            nc.vector.tensor_add(ub[:], ps3[:], cw[bname][:])
                            nc.scalar.copy(ob[:], ub[:])
                        else:
                            nc.scalar.copy(ob[:], ps3[:])
                        nc.sync.dma_start(b.d[dout][lo : lo + nv, :], ob[:nv, :])

            # Preps for the first window BEFORE embeds: GPSIMD starts at t=0.
            n_pre = min(PREP_AHEAD, len(chunks))
            for gi in range(n_pre):
                emit_prep(chunks[gi])

            embed("ve_", "varT_aug", "var_rows", NVLp, VF, v0T,
                  [("vc_wl", "vc_bl", "lp1_loc", NVL, dt.bfloat16),
                   ("cv_wr", None, "rp2_loc", NVL, dt.bfloat16)])
            gp_chain(nc.gpsimd.collective_compute(
                "AllGather", AL.bypass, ins=[lp1_loc[:]], outs=[lp1_full[:]],
                replica_groups=[list(range(NCORES))]))
            embed("ce_", "consT_aug", "cons_rows", NCLp, CF, c0T,
                  [("vc_wr", None, "rp1_loc", NCL, dt.bfloat16)])

            # =========== conv edge-chunk processing ===========
            conv_state = {
                1: dict(pre="vc_", rp_dram=rp1_loc, acc=acc1, n_valid=NCL,
                        oh_d=b.d["e1_oh"], ohT_d=b.d["e1_ohT"], rp_tiles={},
                        cur_ps=[None]),
                2: dict(pre="cv_", rp_dram=rp2_loc, acc=acc2, n_valid=NVL,
                        oh_d=b.d["e2_oh"], ohT_d=b.d["e2_ohT"], rp_tiles={},
                        cur_ps=[None]),
            }

            def get_rp(st, blk):
                if blk in st["rp_tiles"]:
                    return st["rp_tiles"][blk]
                rp_sb = wpool.tile([128, EMB], dt.bfloat16, tag="rpblk")
                lo = blk * 128
                nv = min(128, st["n_valid"] - lo)
                if nv < 128:
                    nc.vector.memset(rp_sb[:], 0.0)
                nc.sync.dma_start(rp_sb[:nv, :], st["rp_dram"][lo : lo + nv, :])
                # wpool recycles rpblk buffers every 3 allocations: keep only
                # the 2 most recent cached handles valid
                st["rp_tiles"][blk] = rp_sb
                while len(st["rp_tiles"]) > 2:
                    st["rp_tiles"].pop(next(iter(st["rp_tiles"])))
                return rp_sb

            def process(ch):
                st = conv_state[ch["conv"]]
                pre = st["pre"]
                fl_triv = p[pre + "fl_trivial"]
                tcn = ch["tcn"]
                t0c = ch["t0"]
                e0 = ch["e0"]
                ne = tcn * 128
                sbuf = ch["g"]
                blk_of_tile = ch["blk_of_tile"]
                acc = st["acc"]
                cur_ps = st["cur_ps"]

                ohe = ohpool.tile([128, CHUNK_TILES * 128], dt.float8e4, tag="ohe")
                nc.sync.dma_start(ohe[:, :ne], st["oh_d"][:, e0 : e0 + ne])
                ohT = ohpool.tile([128, CHUNK_TILES * 128], dt.float8e4, tag="ohT")
                nc.sync.dma_start(ohT[:, :ne], st["ohT_d"][:, e0 : e0 + ne])

                xw_c = xwpool.tile([128, CHUNK_TILES, EMB], dt.bfloat16, tag="xwc")

                gi = 0
                while gi < tcn:
                    gn = min(4, tcn - gi)
                    psg = psA.tile([128, 4, EMB], dt.float32, tag="ps")
                    for k in range(gn):
                        ti = gi + k
                        blk, _, _ = blk_of_tile[t0c + ti]
                        rp_sb = get_rp(st, blk)
                        nc.tensor.matmul(psg[:, k, :],
                                         ohT[:, ti * 128 : (ti + 1) * 128],
                                         rp_sb[:], start=True, stop=True)
                    nc.vector.tensor_tensor(
                        xw_c[:, gi : gi + gn, :], sbuf[:, gi : gi + gn, :],
                        psg[:, :gn, :], AL.add)
                    gi += gn

                # chunk-batched LN stats; the gather tile is dead after the
                # add, so reuse it as square / pair-sum scratch
                sq = sbuf
                nc.vector.tensor_tensor(sq[:, :tcn, :], xw_c[:, :tcn, :],
                                        xw_c[:, :tcn, :], AL.mult)
                sqh = sq[:, :, : EMB // 2]
                nc.vector.tensor_tensor(
                    sqh[:, :tcn, :], sq[:, :tcn, : EMB // 2],
                    sq[:, :tcn, EMB // 2 :], AL.add)
                xh = sq[:, :, EMB // 2 :]
                nc.vector.tensor_tensor(
                    xh[:, :tcn, :], xw_c[:, :tcn, : EMB // 2],
                    xw_c[:, :tcn, EMB // 2 :], AL.add)
                sx = tpool.tile([128, CHUNK_TILES], dt.float32, tag="sxc")
                nc.vector.reduce_sum(sx[:, :tcn], xh[:, :tcn, :],
                                     axis=mybir.AxisListType.X)
                sxx = tpool.tile([128, CHUNK_TILES], dt.float32, tag="sxxc")
                nc.vector.reduce_sum(sxx[:, :tcn], sqh[:, :tcn, :],
                                     axis=mybir.AxisListType.X)
                inv = 1.0 / EMB
                mu = tpool.tile([128, CHUNK_TILES], dt.float32, tag="muc")
                nc.vector.tensor_scalar_mul(mu[:, :tcn], sx[:, :tcn], inv)
                veps = tpool.tile([128, CHUNK_TILES], dt.float32, tag="vepsc")
                nc.vector.tensor_scalar(veps[:, :tcn], sxx[:, :tcn], inv, EPS,
                                        AL.mult, AL.add)
                nmusq = tpool.tile([128, CHUNK_TILES], dt.float32, tag="nmusqc")
                nc.vector.scalar_tensor_tensor(
                    nmusq[:, :tcn], mu[:, :tcn], -1.0, mu[:, :tcn], AL.mult, AL.mult)
                nc.vector.tensor_tensor(veps[:, :tcn], veps[:, :tcn],
                                        nmusq[:, :tcn], AL.add)
                rstd_t = rsqrt_newton(nc, tpool, veps[:, :tcn], tcn, "crs")
                nmr_c = tpool.tile([128, CHUNK_TILES], dt.float32, tag="nmrc")
                nc.vector.scalar_tensor_tensor(
                    nmr_c[:, :tcn], mu[:, :tcn], -1.0, rstd_t, AL.mult, AL.mult)

                for ti in range(tcn):
                    blk, isfirst, islast = blk_of_tile[t0c + ti]
                    act = wpool.tile([128, EMB], dt.bfloat16, tag="act")
                    if fl_triv:
                        nc.scalar.activation(
                            act[:], xw_c[:, ti, :], LR,
                            bias=nmr_c[:, ti : ti + 1],
                            scale=rstd_t[:, ti : ti + 1], alpha=SLOPE)
                    else:
                        y1 = wpool.tile([128, EMB], dt.float32, tag="y1")
                        nc.vector.tensor_scalar(
                            y1[:], xw_c[:, ti, :], mu[:, ti : ti + 1],
                            rstd_t[:, ti : ti + 1], AL.subtract, AL.mult)
                        y2 = wpool.tile([128, EMB], dt.float32, tag="y2")
                        nc.vector.scalar_tensor_tensor(
                            y2[:], y1[:], 1.0, cw[pre + "flg"][:], AL.mult, AL.mult)
                        y3 = wpool.tile([128, EMB], dt.float32, tag="y3")
                        nc.vector.tensor_add(y3[:], y2[:], cw[pre + "flb"][:])
                        nc.scalar.activation(act[:], y3[:], LR,
                                             bias=zero_col[:], alpha=SLOPE)
                    if cur_ps[0] is None:
                        psb_new = psagg.tile([128, EMB], dt.float32, tag="agg")
                        cur_ps[0] = psb_new
                    psb = cur_ps[0]
                    nc.tensor.matmul(
                        psb[:], ohe[:, ti * 128 : (ti + 1) * 128], act[:],
                        start=isfirst, stop=islast)
                    if islast:
                        nc.vector.tensor_add(acc[:, blk, :], acc[:, blk, :], psb[:])
                        cur_ps[0] = None

            # =========== conv post ===========
            def conv_post(cv, pre, acc, rightT, outT, lpout_name, lpout_w, lpout_b,
                          n_valid, cnt_sb, eps_sb):
                nblocks = cv.nblocks
                for g0 in range(0, nblocks, 8):
                    gb = min(8, nblocks - g0)
                    ub = postp.tile([128, 8, EMB], dt.float32, tag="pub")
                    for k in range(gb):
                        blk = g0 + k
                        accT = wpool.tile([128, 128], dt.bfloat16, tag="accT")
                        transpose_to(accT[:], acc[:, blk, :], 128, 128)
                        ps = psA.tile([128, EMB], dt.float32, tag="ps")
                        nc.tensor.matmul(ps[:], accT[:], cw[pre + "wf"][:],
                                         start=True, stop=True)
                        nc.vector.scalar_tensor_tensor(
                            ub[:, k, :], cw[pre + "bf"][:], cnt_sb[:, blk : blk + 1],
                            ps[:], AL.mult, AL.add)
                    psx = tpool.tile([128, 8], dt.float32, tag="psx")
                    nc.vector.reduce_sum(psx[:, :gb], ub[:, :gb, :],
                                         axis=mybir.AxisListType.X)
                    sqg = postp.tile([128, 8, EMB], dt.float32, tag="psqg")
                    nc.vector.tensor_tensor(sqg[:, :gb, :], ub[:, :gb, :],
                                            ub[:, :gb, :], AL.mult)
                    psxx = tpool.tile([128, 8], dt.float32, tag="psxx")
                    nc.vector.reduce_sum(psxx[:, :gb], sqg[:, :gb, :],
                                         axis=mybir.AxisListType.X)
                    inv = 1.0 / EMB
                    pmu = tpool.tile([128, 8], dt.float32, tag="pmu")
                    nc.vector.tensor_scalar_mul(pmu[:, :gb], psx[:, :gb], inv)
                    pveps = tpool.tile([128, 8], dt.float32, tag="pveps")
                    nc.vector.scalar_tensor_tensor(
                        pveps[:, :gb], psxx[:, :gb], inv, eps_sb[:, g0 : g0 + gb],
                        AL.mult, AL.add)
                    pnmusq = tpool.tile([128, 8], dt.float32, tag="pnmusq")
                    nc.vector.scalar_tensor_tensor(
                        pnmusq[:, :gb], pmu[:, :gb], -1.0, pmu[:, :gb],
                        AL.mult, AL.mult)
                    nc.vector.tensor_tensor(pveps[:, :gb], pveps[:, :gb],
                                            pnmusq[:, :gb], AL.add)
                    prstd_t = rsqrt_newton(nc, tpool, pveps[:, :gb], gb, "prs")
                    for k in range(gb):
                        blk = g0 + k
                        lo = blk * 128
                        nv = min(128, n_valid - lo)
                        lnv = wpool.tile([128, EMB], dt.bfloat16, tag="lnv")
                        nc.vector.tensor_scalar(
                            lnv[:], ub[:, k, :], pmu[:, k : k + 1],
                            prstd_t[:, k : k + 1], AL.subtract, AL.mult)
                        lnT = wpool.tile([128, 128], dt.bfloat16, tag="lnT")
                        transpose_to(lnT[:], lnv[:], 128, 128)
                        ps2 = psA.tile([128, EMB], dt.float32, tag="ps")
                        nc.tensor.matmul(ps2[:], lnT[:], cw[pre + "wo1a"][:],
                                         start=True, stop=False)
                        nc.tensor.matmul(ps2[:], rightT[:, lo : lo + 128],
                                         cw[pre + "wo1b"][:], start=False, stop=True)
                        u2 = wpool.tile([128, EMB], dt.float32, tag="pcu2")
                        nc.vector.tensor_add(u2[:], ps2[:], cw[pre + "bo1"][:])
                        tml = wpool.tile([128, EMB], dt.bfloat16, tag="tml")
                        nc.scalar.activation(tml[:], u2[:], LR, bias=zero_col[:],
                                             alpha=SLOPE)
                        tT = wpool.tile([128, 128], dt.bfloat16, tag="tT")
                        transpose_to(tT[:], tml[:], 128, 128)
                        ps3 = psA.tile([128, EMB], dt.float32, tag="ps")
                        nc.tensor.matmul(ps3[:], tT[:], cw[pre + "wo2"][:],
                                         start=True, stop=True)
                        u3 = wpool.tile([128, EMB], dt.float32, tag="pcu3")
                        nc.vector.tensor_add(u3[:], ps3[:], cw[pre + "bo2"][:])
                        res = wpool.tile([128, EMB], dt.bfloat16, tag="res")
                        nc.scalar.copy(res[:], u3[:])
                        transpose_to(outT[:, lo : lo + 128], res[:], 128, 128)
                        if lpout_name is not None and nv > 0:
                            ps4 = psA.tile([128, EMB], dt.float32, tag="ps")
                            nc.tensor.matmul(ps4[:], outT[:, lo : lo + 128],
                                             cw[lpout_w][:], start=True, stop=True)
                            ub4 = wpool.tile([128, EMB], dt.float32, tag="pc4u")
                            nc.vector.tensor_add(ub4[:], ps4[:], cw[lpout_b][:])
                            ob = wpool.tile([128, EMB], dt.bfloat16, tag="pc4o")
                            nc.scalar.copy(ob[:], ub4[:])
                            nc.sync.dma_start(b.d[lpout_name][lo : lo + nv, :],
                                              ob[:nv, :])

            # ---- flattened chunk pipeline ----
            n1 = sum(1 for ch in chunks if ch["conv"] == 1)
            for k, ch in enumerate(chunks):
                if ch["conv"] == 2 and k == n1:
                    # conv1 done: post + AllGather lp2 (GPSIMD keeps prepping
                    # conv2 chunks already in the window)
                    conv_post(c1p, "vc_", acc1, c0T, c1T, "lp2_loc", "cv_wl",
                              "cv_bl", NCL, cnt1, eps1)
                    gp_chain(nc.gpsimd.collective_compute(
                        "AllGather", AL.bypass, ins=[lp2_loc[:]],
                        outs=[lp2_full[:]], replica_groups=[list(range(NCORES))]))
                emit_trigger(ch)
                if k + PREP_AHEAD < len(chunks):
                    emit_prep(chunks[k + PREP_AHEAD])
                process(ch)

            conv_post(c2p, "cv_", acc2, v0T, v1T, None, None, None,
                      NVL, cnt2, eps2)

            # =========== heads ===========
            if nact == 0:
                zrow = wpool.tile([1, 512], dt.bfloat16, tag="orow")
                nc.vector.memset(zrow[:], 0.0)
                for j in range(NVLh // 512):
                    nc.sync.dma_start(out_d[:, j * 512 : (j + 1) * 512], zrow[:])
            else:
                nch = NVLh // 512
                for j in range(nch):
                    pso = psout.tile([1, 512], dt.float32, tag="pso")
                    for hi in range(nact):
                        hw1t = wpool.tile([128, 128], dt.bfloat16, tag="hw1t")
                        nc.sync.dma_start(hw1t[:], b.d["hw1"][:, hi, :])
                        ps = psA.tile([128, 512], dt.float32, tag="ps")
                        nc.tensor.matmul(ps[:], hw1t[:],
                                         v1T[:, j * 512 : (j + 1) * 512],
                                         start=True, stop=True)
                        hh = wpool.tile([128, 512], dt.bfloat16, tag="hh")
                        if hi % 7 < 2:
                            zt = headp.tile([128, 512], dt.float32, tag="hzt")
                            nc.vector.tensor_scalar(
                                zt[:], ps[:], cw["hb1"][:, hi : hi + 1], None,
                                AL.add)
                            st2 = headp.tile([128, 512], dt.bfloat16, tag="hst")
                            nc.vector.tensor_scalar_mul(st2[:], zt[:], SLOPE)
                            nc.vector.tensor_tensor(hh[:], zt[:], st2[:], AL.max)
                        else:
                            nc.scalar.activation(hh[:], ps[:], LR,
                                                 bias=cw["hb1"][:, hi : hi + 1],
                                                 scale=1.0, alpha=SLOPE)
                        nc.tensor.matmul(pso[:], cw["hw2"][:, hi : hi + 1], hh[:],
                                         start=(hi == 0), stop=(hi == nact - 1))
                    orow = cpool.tile([1, 512], dt.bfloat16, tag="orow")
                    nc.scalar.copy(orow[:], pso[:])
                    nc.sync.dma_start(out_d[:, j * 512 : (j + 1) * 512], orow[:])

    nc.compile()
    return b


_CACHE = {}


def kernel(**inputs):
    key = tuple(sorted((k, tuple(np.asarray(v).shape)) for k, v in inputs.items()))
    p = host_prep(inputs)
    ck = (key, p["nact"], p["conv1"].etot, p["conv2"].etot,
          p["vc_fl_trivial"], p["cv_fl_trivial"])
    if ck in _CACHE:
        b = _CACHE[ck]
    else:
        b = build_program(p)
        _CACHE[ck] = b
    in_maps = [dict(p["core_inputs"][c]) for c in range(NCORES)]
    res = run_bass_kernel_spmd(b.nc, in_maps, core_ids=list(range(NCORES)))
    NVL = p["NVL"]
    out = np.concatenate([res.results[c]["out"][0, :NVL] for c in range(NCORES)])
    out = out.astype(np.float32) * p["out_scale"] + p["out_add"]
    return out.astype(np.float32)


# revision 12
# speedup vs baseline: 1.0128x; 1.0128x over previous
"""Trainium2 Bass kernel for BipartiteGCN (8 NeuronCores, SPMD). v2

Strategy (v2 — scheduling-first rewrite):
 - Node rows sharded 8 ways; edges sharded by DESTINATION range, sorted by
   dst block then src; per-edge lp[src] rows fetched via SWDGE dma_gather.
 - SWDGE descriptor generation (~7.7ns/row, ~2.0ms/core total) is the hard
   floor; everything else must overlap it:
     * gathers issued as prepare_only descriptor preps with a W-deep
       sliding window flattened across BOTH convs; triggers fire when the
       lp tables land. GPSIMD starts prepping at t=0 (during embeds) and
       never idles.
     * pools are deep enough that DVE stats of chunk k never WAR-wait on
       gather k-4 completions (the v1 failure mode: 1.7ms of DVE stalls).
 - Scatter-mean division eliminated exactly: LN is scale-invariant per row,
   so LN(agg/cnt + bf) == LN(agg + cnt*bf) with eps -> eps*max(cnt,1)^2
   (cnt is host-known index data, shipped as per-partition constants).
 - All rsqrt on DVE via Newton iteration (no ACT Sqrt table churn).
 - Embed / conv-post / heads as v1 otherwise.
"""

import os
import sys

for _p in ("/opt/trn_rl_repo",):
    if _p not in sys.path:
        sys.path.insert(0, _p)

import numpy as np
import ml_dtypes

import concourse.bass as bass
import concourse.bacc as bacc
import concourse.mybir as mybir
from concourse import tile, library_config
from concourse.bass_utils import run_bass_kernel_spmd
from concourse import hw_specs as _hw_specs

# Calibrated SWDGE dma_gather descriptor-generation rate (measured ~7.7
# ns/descriptor on HW; the stock 0.34 makes the Tile scheduler mis-order).
_hw_specs.TRN2Spec.SWDGE_NS_PER_DESCRIPTOR = 7.7

BF16 = ml_dtypes.bfloat16
F32 = np.float32
NCORES = 8
EMB = 128
CHUNK_TILES = 20     # tiles (128 edges) per dma_gather
PREP_AHEAD = 4       # gather chunks prepped ahead of their trigger
PREP_MODE = os.environ.get("KPREP", "prep")  # "prep" | "inline"
HI_BASE = 32768
EPS = 1e-5
SLOPE = 0.01

dt = mybir.dt


def _wrap_idx(idx_i16):
    """[N] int16 -> [128, N//16] wrapped (i at [i%16, i//16]) + replicated 8x."""
    n = idx_i16.shape[0]
    assert n % 16 == 0
    w = idx_i16.reshape(n // 16, 16).T
    return np.tile(w, (8, 1)).copy()


def _bcast_row(v, rows=128):
    return np.broadcast_to(np.asarray(v, F32)[None, :], (rows, v.shape[0])).copy()


class ConvPrep:
    """Per-conv edge-sharding data. Same segment layout for all cores."""

    def __init__(self, dst, src, n_dst, n_src, dst_per_core):
        self.n_dst_local = dst_per_core
        self.nblocks = -(-dst_per_core // 128)
        nb = self.nblocks
        self.two_buckets = n_src > HI_BASE
        nu = 2 if self.two_buckets else 1
        self.nu = nu

        core = dst // dst_per_core
        dloc_all = dst - core * dst_per_core
        block_all = dloc_all // 128

        per = [[[None] * nb for _ in range(nu)] for _ in range(NCORES)]
        for c in range(NCORES):
            m = core == c
            d_c = dloc_all[m]
            s_c = src[m]
            b_c = block_all[m]
            u_c = (s_c >= HI_BASE).astype(np.int8) if self.two_buckets else np.zeros(
                len(s_c), np.int8
            )
            for u in range(nu):
                mu = u_c == u
                db, sb, bb = d_c[mu], s_c[mu], b_c[mu]
                order = np.argsort(bb, kind="stable")
                db, sb, bb = db[order], sb[order], bb[order]
                bounds = np.searchsorted(bb, np.arange(nb + 1))
                for b in range(nb):
                    lo, hi = bounds[b], bounds[b + 1]
                    o2 = np.argsort(sb[lo:hi], kind="stable")
                    per[c][u][b] = (sb[lo:hi][o2], db[lo:hi][o2])

        self.ntiles = np.zeros((nu, nb), np.int64)
        for u in range(nu):
            for b in range(nb):
                mx = max(len(per[c][u][b][0]) for c in range(NCORES))
                self.ntiles[u, b] = -(-mx // 128) if mx > 0 else 0

        etot = int(self.ntiles.sum()) * 128
        self.etot = etot

        # host-known scatter-mean counts: cnt and EPS*max(cnt,1)^2
        self.cnt = np.zeros((NCORES, 128, nb), F32)
        self.eps2 = np.zeros((NCORES, 128, nb), F32)
        for c in range(NCORES):
            cnt = np.bincount(dloc_all[core == c], minlength=nb * 128).astype(F32)
            cnt = cnt[: nb * 128].reshape(nb, 128).T  # [128, nb]
            self.cnt[c] = cnt
            self.eps2[c] = EPS * np.maximum(cnt, 1.0) ** 2

        self.src_idx = np.zeros((NCORES, etot), np.int16)
        self.dstrel = np.full((NCORES, etot), -1.0, F32)
        off = 0
        self.seg_offsets = {}
        for u in range(nu):
            for b in range(nb):
                g = int(self.ntiles[u, b])
                if g == 0:
                    continue
                self.seg_offsets[(u, b)] = off
                for c in range(NCORES):
                    sb, db = per[c][u][b]
                    n = len(sb)
                    s_adj = sb - (HI_BASE if u == 1 else 0)
                    self.src_idx[c, off : off + n] = s_adj.astype(np.int16)
                    self.dstrel[c, off : off + n] = (db - 128 * b).astype(F32)
                off += g * 128
        assert off == etot

        self.streams = []
        for u in range(nu):
            blocks = [(b, int(self.ntiles[u, b])) for b in range(nb) if self.ntiles[u, b] > 0]
            start = self.seg_offsets[(u, blocks[0][0])] if blocks else 0
            nt = sum(g for _, g in blocks)
            self.streams.append({"u": u, "blocks": blocks, "start_edge": start, "ntiles": nt})

    def core_arrays(self, c):
        dr = self.dstrel[c]
        i = np.nonzero(dr >= 0)[0]
        lane = i % 128
        tb = (i // 128) * 128
        d = dr[i].astype(np.int64)
        oh = np.zeros((128, self.etot), ml_dtypes.float8_e4m3)
        oh[lane, tb + d] = 1.0
        ohT = np.zeros((128, self.etot), ml_dtypes.float8_e4m3)
        ohT[d, tb + lane] = 1.0
        return _wrap_idx(self.src_idx[c]), oh, ohT


def host_prep(inputs):
    p = {}
    cons_x = np.asarray(inputs["cons_x"], F32)
    var_x = np.asarray(inputs["var_x"], F32)
    edge_cons = np.asarray(inputs["edge_cons"]).astype(np.int64)
    edge_var = np.asarray(inputs["edge_var"]).astype(np.int64)
    head_mask = np.asarray(inputs["head_mask"]).astype(bool)

    NC, CF = cons_x.shape
    NV, VF = var_x.shape
    assert NC % NCORES == 0 and NV % NCORES == 0
    NCL, NVL = NC // NCORES, NV // NCORES
    p.update(NC=NC, NV=NV, CF=CF, VF=VF, NCL=NCL, NVL=NVL)

    p["conv1"] = ConvPrep(edge_cons, edge_var, NC, NV, NCL)
    p["conv2"] = ConvPrep(edge_var, edge_cons, NV, NC, NVL)

    w = {}

    def embed_w(prefix, g, b, w1, b1, w2, b2, feat):
        w1 = np.asarray(w1, F32)
        w1g = np.asarray(g, F32)[:, None] * w1
        aug = np.concatenate([w1g, np.zeros((1, w1.shape[1]), F32)], 0)
        w[prefix + "w1aug"] = aug.astype(BF16)
        w[prefix + "s1"] = _bcast_row(np.asarray(b, F32) @ w1 + np.asarray(b1, F32))
        w[prefix + "r1"] = _bcast_row(w1g.sum(0))
        w[prefix + "w2"] = np.asarray(w2, F32).astype(BF16)
        w[prefix + "b2"] = _bcast_row(np.asarray(b2, F32))

    embed_w("ce_", inputs["ce_ln_g"], inputs["ce_ln_b"], inputs["ce_w1"],
            inputs["ce_b1"], inputs["ce_w2"], inputs["ce_b2"], CF)
    embed_w("ve_", inputs["ve_ln_g"], inputs["ve_ln_b"], inputs["ve_w1"],
            inputs["ve_b1"], inputs["ve_w2"], inputs["ve_b2"], VF)

    for pre in ("vc_", "cv_"):
        wl = np.asarray(inputs[pre + "wl"], F32)
        w[pre + "wl"] = wl.astype(BF16)
        w[pre + "bl"] = _bcast_row(np.asarray(inputs[pre + "bl"], F32))
        w[pre + "wr"] = np.asarray(inputs[pre + "wr"], F32).astype(BF16)
        flg = np.asarray(inputs[pre + "flg"], F32)
        flb = np.asarray(inputs[pre + "flb"], F32)
        p[pre + "fl_trivial"] = bool(np.all(flg == 1.0) and np.all(flb == 0.0))
        w[pre + "flg"] = _bcast_row(flg)
        w[pre + "flb"] = _bcast_row(flb)
        w[pre + "wf"] = np.asarray(inputs[pre + "wf"], F32).astype(BF16)
        w[pre + "bf"] = _bcast_row(np.asarray(inputs[pre + "bf"], F32))
        wo1 = np.asarray(inputs[pre + "wo1"], F32)
        plg = np.asarray(inputs[pre + "plg"], F32)
        plb = np.asarray(inputs[pre + "plb"], F32)
        w[pre + "wo1a"] = (plg[:, None] * wo1[:EMB]).astype(BF16)
        w[pre + "wo1b"] = wo1[EMB:].astype(BF16)
        w[pre + "bo1"] = _bcast_row(np.asarray(inputs[pre + "bo1"], F32) + plb @ wo1[:EMB])
        w[pre + "wo2"] = np.asarray(inputs[pre + "wo2"], F32).astype(BF16)
        w[pre + "bo2"] = _bcast_row(np.asarray(inputs[pre + "bo2"], F32))

    active = np.nonzero(head_mask)[0]
    nact = int(len(active))
    p["nact"] = nact
    denom = max(float(head_mask.sum()), 1.0)
    hb2 = np.asarray(inputs["hb2"], F32)
    p["out_scale"] = 1.0 / denom
    p["out_add"] = float(hb2[active].sum() / denom)
    if nact > 0:
        hw1 = np.asarray(inputs["hw1"], F32)[active]
        w["hw1"] = hw1.transpose(1, 0, 2).astype(BF16).copy()
        w["hb1"] = np.asarray(inputs["hb1"], F32)[active].T.copy()
        w["hw2"] = np.asarray(inputs["hw2"], F32)[active].T.astype(BF16).copy()

    w["identity"] = np.eye(128, dtype=BF16)
    p["weights"] = w

    NCLp = -(-NCL // 128) * 128
    NVLp = -(-NVL // 128) * 128
    p.update(NCLp=NCLp, NVLp=NVLp)
    NVLh = -(-NVL // 512) * 512
    p["NVLh"] = NVLh

    core_inputs = []
    for c in range(NCORES):
        m = {}
        cx = cons_x[c * NCL : (c + 1) * NCL]
        vx = var_x[c * NVL : (c + 1) * NVL]
        cxp = np.zeros((NCLp, CF), F32)
        cxp[:NCL] = cx
        vxp = np.zeros((NVLp, VF), F32)
        vxp[:NVL] = vx
        m["cons_rows"] = cxp.reshape(NCLp // 128, 128, CF).transpose(1, 0, 2).copy()
        m["var_rows"] = vxp.reshape(NVLp // 128, 128, VF).transpose(1, 0, 2).copy()
        m["consT_aug"] = np.concatenate([cxp.T, np.ones((1, NCLp), F32)], 0).astype(BF16)
        m["varT_aug"] = np.concatenate([vxp.T, np.ones((1, NVLp), F32)], 0).astype(BF16)
        s1, oh1, ohT1 = p["conv1"].core_arrays(c)
        m["e1_src"], m["e1_oh"], m["e1_ohT"] = s1, oh1, ohT1
        s2, oh2, ohT2 = p["conv2"].core_arrays(c)
        m["e2_src"], m["e2_oh"], m["e2_ohT"] = s2, oh2, ohT2
        m["e1_cnt"] = p["conv1"].cnt[c]
        m["e1_eps"] = p["conv1"].eps2[c]
        m["e2_cnt"] = p["conv2"].cnt[c]
        m["e2_eps"] = p["conv2"].eps2[c]
        for k, v in w.items():
            m[k] = v
        core_inputs.append(m)
    p["core_inputs"] = core_inputs
    return p


# ---------------------------------------------------------------------------


class B:
    def __init__(self, p):
        self.p = p
        self.nc = bacc.Bacc("TRN2", target_bir_lowering=False, debug=False,
                            num_devices=NCORES, num_swdge_queues=2)
        self.d = {}

    def dram(self, name, shape, dtype, kind=None, addr_space=None):
        kw = {}
        if kind:
            kw["kind"] = kind
        if addr_space:
            kw["addr_space"] = addr_space
        t = self.nc.dram_tensor(name, list(shape), dtype, **kw)
        self.d[name] = t
        return t


MAGIC = 0x5F3759DF


def rsqrt_newton(nc, pool, src_ap, n, tag, iters=2):
    """1/sqrt(src) on DVE only. src_ap [128, n] f32 > 0."""
    AL = mybir.AluOpType
    sh = pool.tile([128, n], dt.int32, tag=tag + "sh")
    nc.vector.tensor_scalar(sh[:], src_ap.bitcast(dt.int32), 1, None,
                            AL.arith_shift_right)
    y0 = pool.tile([128, n], dt.int32, tag=tag + "y0")
    nc.vector.tensor_scalar(y0[:], sh[:], -1, MAGIC, AL.mult, AL.add)
    cur = y0[:].bitcast(dt.float32)
    h = pool.tile([128, n], dt.float32, tag=tag + "h")
    nc.vector.tensor_scalar_mul(h[:], src_ap, 0.5)
    for it in range(iters):
        yy = pool.tile([128, n], dt.float32, tag=tag + f"yy{it}")
        nc.vector.tensor_tensor(yy[:], cur, cur, AL.mult)
        nc.vector.tensor_tensor(yy[:], yy[:], h[:], AL.mult)
        nc.vector.tensor_scalar(yy[:], yy[:], -1.0, 1.5, AL.mult, AL.add)
        nxt = pool.tile([128, n], dt.float32, tag=tag + f"n{it}")
        nc.vector.tensor_tensor(nxt[:], cur, yy[:], AL.mult)
        cur = nxt[:]
    return cur


def build_program(p):
    b = B(p)
    nc = b.nc
    w = p["weights"]
    NCL, NVL, NCLp, NVLp = p["NCL"], p["NVL"], p["NCLp"], p["NVLp"]
    CF, VF = p["CF"], p["VF"]
    NC, NV = p["NC"], p["NV"]
    NVLh = p["NVLh"]
    nact = p["nact"]

    din = lambda n, s, t: b.dram(n, s, t, kind="ExternalInput")
    din("cons_rows", [128, NCLp // 128, CF], dt.float32)
    din("var_rows", [128, NVLp // 128, VF], dt.float32)
    din("consT_aug", [CF + 1, NCLp], dt.bfloat16)
    din("varT_aug", [VF + 1, NVLp], dt.bfloat16)
    c1p, c2p = p["conv1"], p["conv2"]
    din("e1_src", [128, c1p.etot // 16], dt.int16)
    din("e1_oh", [128, c1p.etot], dt.float8e4)
    din("e1_ohT", [128, c1p.etot], dt.float8e4)
    din("e2_src", [128, c2p.etot // 16], dt.int16)
    din("e2_oh", [128, c2p.etot], dt.float8e4)
    din("e2_ohT", [128, c2p.etot], dt.float8e4)
    din("e1_cnt", [128, c1p.nblocks], dt.float32)
    din("e1_eps", [128, c1p.nblocks], dt.float32)
    din("e2_cnt", [128, c2p.nblocks], dt.float32)
    din("e2_eps", [128, c2p.nblocks], dt.float32)
    for k, v in w.items():
        dtt = dt.bfloat16 if v.dtype == BF16 else (dt.int16 if v.dtype == np.int16 else dt.float32)
        din(k, list(v.shape), dtt)
    out_d = b.dram("out", [1, NVLh], dt.bfloat16, kind="ExternalOutput")

    lp1_loc = b.dram("lp1_loc", [NVL, EMB], dt.bfloat16)
    lp1_full = b.dram("lp1_full", [NV, EMB], dt.bfloat16, addr_space="Shared")
    rp1_loc = b.dram("rp1_loc", [NCL, EMB], dt.bfloat16)
    lp2_loc = b.dram("lp2_loc", [NCL, EMB], dt.bfloat16)
    lp2_full = b.dram("lp2_full", [NC, EMB], dt.bfloat16, addr_space="Shared")
    rp2_loc = b.dram("rp2_loc", [NVL, EMB], dt.bfloat16)

    LR = mybir.ActivationFunctionType.Lrelu
    AL = mybir.AluOpType

    with tile.TileContext(nc) as tc:
        nc.gpsimd.load_library(library_config.mlp)
        from contextlib import ExitStack
        with ExitStack() as _stack:
            _ep = _stack.enter_context
            cpool = _ep(tc.tile_pool(name="const", bufs=1))
            rpool = _ep(tc.tile_pool(name="resident", bufs=1))
            wpool = _ep(tc.tile_pool(name="work", bufs=3))
            tpool = _ep(tc.tile_pool(name="tiny", bufs=5))
            gpool = _ep(tc.tile_pool(name="gath", bufs=PREP_AHEAD + 1))
            sidxpool = _ep(tc.tile_pool(name="sidxp", bufs=PREP_AHEAD + 2))
            ohpool = _ep(tc.tile_pool(name="ohp", bufs=2))
            xwpool = _ep(tc.tile_pool(name="xwp", bufs=2))
            embp = _ep(tc.tile_pool(name="embp", bufs=1))
            sqpool = _ep(tc.tile_pool(name="sqp", bufs=2))
            hw1pool = _ep(tc.tile_pool(name="hw1p", bufs=8))
            postp = _ep(tc.tile_pool(name="postp", bufs=2))
            headp = _ep(tc.tile_pool(name="headp", bufs=2))
            psA = _ep(tc.tile_pool(name="psA", bufs=4, space="PSUM"))
            psT = _ep(tc.tile_pool(name="psT", bufs=1, space="PSUM"))
            psagg = _ep(tc.tile_pool(name="psagg", bufs=2, space="PSUM"))
            psout = _ep(tc.tile_pool(name="psout", bufs=1, space="PSUM"))
            # ---- constants ----
            cw = {}
            for k, v in w.items():
                if k == "hw1":
                    continue  # streamed during the head stage
                dtt = dt.bfloat16 if v.dtype == BF16 else dt.float32
                t = cpool.tile(list(v.shape), dtt, tag=k)
                nc.sync.dma_start(t[:], b.d[k][:])
                cw[k] = t

            ident = cw["identity"]
            zero_col = cpool.tile([128, 1], dt.float32, tag="zero_col")
            nc.vector.memset(zero_col[:], 0.0)

            cnt1 = cpool.tile([128, c1p.nblocks], dt.float32, tag="cnt1")
            nc.sync.dma_start(cnt1[:], b.d["e1_cnt"][:])
            eps1 = cpool.tile([128, c1p.nblocks], dt.float32, tag="eps1")
            nc.sync.dma_start(eps1[:], b.d["e1_eps"][:])
            cnt2 = cpool.tile([128, c2p.nblocks], dt.float32, tag="cnt2")
            nc.sync.dma_start(cnt2[:], b.d["e2_cnt"][:])
            eps2 = cpool.tile([128, c2p.nblocks], dt.float32, tag="eps2")
            nc.sync.dma_start(eps2[:], b.d["e2_eps"][:])

            # residents
            c0T = rpool.tile([128, NCLp], dt.bfloat16, tag="c0T")
            v0T = rpool.tile([128, NVLp], dt.bfloat16, tag="v0T")
            c1T = rpool.tile([128, NCLp], dt.bfloat16, tag="c1T")
            v1T = rpool.tile([128, NVLh], dt.bfloat16, tag="v1T")
            nc.vector.memset(v1T[:], 0.0)
            acc1 = rpool.tile([128, c1p.nblocks, EMB], dt.bfloat16, tag="acc1")
            nc.vector.memset(acc1[:], 0.0)
            acc2 = rpool.tile([128, c2p.nblocks, EMB], dt.bfloat16, tag="acc2")
            nc.vector.memset(acc2[:], 0.0)

            # ---- SWDGE prep/trigger machinery ----
            dma_sems = [nc.alloc_semaphore("gq0"), nc.alloc_semaphore("gq1")]
            prep_sem = nc.alloc_semaphore("prep_ctr")
            gp_last = [None]

            def gp_chain(inst):
                if gp_last[0] is not None:
                    deps = bass.InstructionNameOrderedSet()
                    deps.add(gp_last[0])
                    inst.ins.add_nosync_dependencies_from(deps)
                gp_last[0] = inst.ins.name
                return inst

            # flattened chunk list across conv1 then conv2
            chunks = []
            for conv_id, cv, lp_dram, src_d in (
                (1, c1p, lp1_full, b.d["e1_src"]),
                (2, c2p, lp2_full, b.d["e2_src"]),
            ):
                for stream in cv.streams:
                    base_edge = stream["start_edge"]
                    ntiles = stream["ntiles"]
                    view_lo = HI_BASE if stream["u"] == 1 else 0
                    blk_of_tile = {}
                    t0 = 0
                    for (blk, tcnt) in stream["blocks"]:
                        for t in range(t0, t0 + tcnt):
                            blk_of_tile[t] = (blk, t == t0, t == t0 + tcnt - 1)
                        t0 += tcnt
                    tdone = 0
                    while tdone < ntiles:
                        tcn = min(CHUNK_TILES, ntiles - tdone)
                        chunks.append(dict(
                            conv=conv_id, lp=lp_dram, src_d=src_d,
                            view_lo=view_lo, e0=base_edge + tdone * 128,
                            t0=tdone, tcn=tcn, blk_of_tile=blk_of_tile,
                        ))
                        tdone += tcn
            for gi, ch in enumerate(chunks):
                ch["q"] = gi % 2

            prep_count = [0]

            def emit_prep(ch):
                ne = ch["tcn"] * 128
                e0 = ch["e0"]
                sidx = sidxpool.tile([128, CHUNK_TILES * 8], dt.int16, tag="sidx")
                nc.sync.dma_start(sidx[:, : ne // 16],
                                  ch["src_d"][:, e0 // 16 : (e0 + ne) // 16])
                g = gpool.tile([128, CHUNK_TILES, EMB], dt.bfloat16, tag="sgat")
                ch["g"] = g
                ch["sidx"] = sidx
                if PREP_MODE != "prep":
                    return
                lp_view = ch["lp"][ch["view_lo"]:, :] if ch["view_lo"] else ch["lp"][:, :]
                inst = nc.gpsimd.dma_gather(
                    g[:, : ch["tcn"], :], lp_view, sidx[:, : ne // 16], ne, ne,
                    EMB, single_packet=False, prepare_only=True,
                    sem=dma_sems[ch["q"]], queue_num=ch["q"])
                inst.then_inc(prep_sem, 1)
                gp_chain(inst)
                prep_count[0] += 1
                ch["prep_no"] = prep_count[0]

            def emit_trigger(ch):
                if PREP_MODE != "prep":
                    ne = ch["tcn"] * 128
                    lp_view = (ch["lp"][ch["view_lo"]:, :] if ch["view_lo"]
                               else ch["lp"][:, :])
                    gp_chain(nc.gpsimd.dma_gather(
                        ch["g"][:, : ch["tcn"], :], lp_view,
                        ch["sidx"][:, : ne // 16], ne, ne, EMB,
                        single_packet=False, queue_num=ch["q"]))
                    return
                gp_chain(nc.gpsimd.wait_ge(prep_sem, ch["prep_no"]))
                gp_chain(nc.gpsimd.trigger_dma(count=1, queue_num=ch["q"]))

            # =========== embeddings ===========
            def transpose_to(dst_ap, src_ap, n_p, n_f):
                ps = psT.tile([128, 128], dt.bfloat16, tag="psT")
                nc.tensor.transpose(ps[:n_f, :n_p], src_ap, ident[:n_p, :n_p])
                nc.scalar.copy(dst_ap, ps[:n_f, :n_p])

            def embed(pre, xT_aug_name, rows_name, nrows_p, nfeat, outT, extra):
                nchunks = nrows_p // 128
                xall = embp.tile([128, nchunks, nfeat], dt.float32, tag="embx")
                nc.sync.dma_start(xall[:], b.d[rows_name][:])
                sx = tpool.tile([128, nchunks], dt.float32, tag="esx")
                nc.vector.reduce_sum(sx[:], xall[:], axis=mybir.AxisListType.X)
                nc.vector.tensor_tensor(xall[:], xall[:], xall[:], AL.mult)
                sxx = tpool.tile([128, nchunks], dt.float32, tag="esxx")
                nc.vector.reduce_sum(sxx[:], xall[:], axis=mybir.AxisListType.X)
                inv = 1.0 / nfeat
                mu_b = tpool.tile([128, nchunks], dt.float32, tag="emub")
                nc.vector.tensor_scalar_mul(mu_b[:], sx[:], inv)
                veps = tpool.tile([128, nchunks], dt.float32, tag="evep")
                nc.vector.tensor_scalar(veps[:], sxx[:], inv, EPS, AL.mult, AL.add)
                nmusq = tpool.tile([128, nchunks], dt.float32, tag="enmu")
                nc.vector.scalar_tensor_tensor(
                    nmusq[:], mu_b[:], -1.0, mu_b[:], AL.mult, AL.mult)
                nc.vector.tensor_tensor(veps[:], veps[:], nmusq[:], AL.add)
                rstd_b = rsqrt_newton(nc, tpool, veps[:], nchunks, "erst")
                nrstd_b = tpool.tile([128, nchunks], dt.float32, tag="enrs")
                nc.vector.tensor_scalar_mul(nrstd_b[:], rstd_b, -1.0)
                for chn in range(nchunks):
                    xTa = wpool.tile([nfeat + 1, 128], dt.bfloat16, tag="xTa")
                    nc.sync.dma_start(xTa[:], b.d[xT_aug_name][:, chn * 128 : (chn + 1) * 128])
                    ps = psA.tile([128, EMB], dt.float32, tag="ps")
                    nc.tensor.matmul(ps[:], xTa[:],
                                     cw[pre + "w1aug"][:], start=True, stop=True)
                    tmid = wpool.tile([128, EMB], dt.float32, tag="embmid")
                    nc.vector.scalar_tensor_tensor(
                        tmid[:], cw[pre + "r1"][:], mu_b[:, chn : chn + 1], ps[:],
                        AL.mult, AL.subtract)
                    tmid2 = wpool.tile([128, EMB], dt.float32, tag="tmid2")
                    nc.vector.scalar_tensor_tensor(
                        tmid2[:], tmid[:], nrstd_b[:, chn : chn + 1], cw[pre + "s1"][:],
                        AL.mult, AL.add)
                    z1 = wpool.tile([128, EMB], dt.bfloat16, tag="z1")
                    nc.scalar.activation(z1[:], tmid2[:], LR, bias=zero_col[:], alpha=SLOPE)
                    z1T = wpool.tile([128, 128], dt.bfloat16, tag="z1T")
                    transpose_to(z1T[:], z1[:], 128, 128)
                    ps2 = psA.tile([128, EMB], dt.float32, tag="ps")
                    nc.tensor.matmul(ps2[:], z1T[:], cw[pre + "w2"][:], start=True, stop=True)
                    u = wpool.tile([128, EMB], dt.float32, tag="embu")
                    nc.vector.tensor_add(u[:], ps2[:], cw[pre + "b2"][:])
                    z2 = wpool.tile([128, EMB], dt.bfloat16, tag="z2")
                    nc.scalar.activation(z2[:], u[:], LR, bias=zero_col[:], alpha=SLOPE)
                    transpose_to(outT[:, chn * 128 : (chn + 1) * 128], z2[:], 128, 128)
                    for (wname, bname, dout, n_valid, odt) in extra:
                        lo = chn * 128
                        nv = min(128, max(0, n_valid - lo))
                        if nv == 0:
                            continue
                        ps3 = psA.tile([128, EMB], dt.float32, tag="ps")
                        nc.tensor.matmul(ps3[:], outT[:, lo : lo + 128],
                                         cw[wname][:], start=True, stop=True)
                        ob = wpool.tile([128, EMB], odt, tag="projo")
                        if bname is not None:
                            ub = wpool.tile([128, EMB], dt.float32, tag="proju")
                            nc.vector.tensor_add(ub[:], ps3[:], cw[bname][:])
                            nc.scalar.copy(ob[:], ub[:])
                        else:
                            nc.scalar.copy(ob[:], ps3[:])
                        nc.sync.dma_start(b.d[dout][lo : lo + nv, :], ob[:nv, :])

            # Preps for the first window BEFORE embeds: GPSIMD starts at t=0.
            n_pre = min(PREP_AHEAD, len(chunks))
            for gi in range(n_pre):
                emit_prep(chunks[gi])

            embed("ve_", "varT_aug", "var_rows", NVLp, VF, v0T,
                  [("vc_wl", "vc_bl", "lp1_loc", NVL, dt.bfloat16),
                   ("cv_wr", None, "rp2_loc", NVL, dt.bfloat16)])
            gp_chain(nc.gpsimd.collective_compute(
                "AllGather", AL.bypass, ins=[lp1_loc[:]], outs=[lp1_full[:]],
                replica_groups=[list(range(NCORES))]))
            embed("ce_", "consT_aug", "cons_rows", NCLp, CF, c0T,
                  [("vc_wr", None, "rp1_loc", NCL, dt.bfloat16)])

            # =========== conv edge-chunk processing ===========
            conv_state = {
                1: dict(pre="vc_", rp_dram=rp1_loc, acc=acc1, n_valid=NCL,
                        oh_d=b.d["e1_oh"], ohT_d=b.d["e1_ohT"], rp_tiles={},
                        cur_ps=[None]),
                2: dict(pre="cv_", rp_dram=rp2_loc, acc=acc2, n_valid=NVL,
                        oh_d=b.d["e2_oh"], ohT_d=b.d["e2_ohT"], rp_tiles={},
                        cur_ps=[None]),
            }

            def get_rp(st, blk):
                if blk in st["rp_tiles"]:
                    return st["rp_tiles"][blk]
                rp_sb = wpool.tile([128, EMB], dt.bfloat16, tag="rpblk")
                lo = blk * 128
                nv = min(128, st["n_valid"] - lo)
                if nv < 128:
                    nc.vector.memset(rp_sb[:], 0.0)
                nc.sync.dma_start(rp_sb[:nv, :], st["rp_dram"][lo : lo + nv, :])
                # wpool recycles rpblk buffers every 3 allocations: keep only
                # the 2 most recent cached handles valid
                st["rp_tiles"][blk] = rp_sb
                while len(st["rp_tiles"]) > 2:
                    st["rp_tiles"].pop(next(iter(st["rp_tiles"])))
                return rp_sb

            def process(ch):
                st = conv_state[ch["conv"]]
                pre = st["pre"]
                fl_triv = p[pre + "fl_trivial"]
                tcn = ch["tcn"]
                t0c = ch["t0"]
                e0 = ch["e0"]
                ne = tcn * 128
                sbuf = ch["g"]
                blk_of_tile = ch["blk_of_tile"]
                acc = st["acc"]
                cur_ps = st["cur_ps"]

                ohe = ohpool.tile([128, CHUNK_TILES * 128], dt.float8e4, tag="ohe")
                nc.sync.dma_start(ohe[:, :ne], st["oh_d"][:, e0 : e0 + ne])
                ohT = ohpool.tile([128, CHUNK_TILES * 128], dt.float8e4, tag="ohT")
                nc.sync.dma_start(ohT[:, :ne], st["ohT_d"][:, e0 : e0 + ne])

                xw_c = xwpool.tile([128, CHUNK_TILES, EMB], dt.bfloat16, tag="xwc")

                gi = 0
                while gi < tcn:
                    gn = min(4, tcn - gi)
                    psg = psA.tile([128, 4, EMB], dt.float32, tag="ps")
                    for k in range(gn):
                        ti = gi + k
                        blk, _, _ = blk_of_tile[t0c + ti]
                        rp_sb = get_rp(st, blk)
                        nc.tensor.matmul(psg[:, k, :],
                                         ohT[:, ti * 128 : (ti + 1) * 128],
                                         rp_sb[:], start=True, stop=True)
                    nc.vector.tensor_tensor(
                        xw_c[:, gi : gi + gn, :], sbuf[:, gi : gi + gn, :],
                        psg[:, :gn, :], AL.add)
                    gi += gn

                # chunk-batched LN stats (sq scratch separate from the gather
                # buffer so the gather ring frees right after the add)
                sq = sqpool.tile([128, CHUNK_TILES, EMB], dt.bfloat16, tag="sqc")
                nc.vector.tensor_tensor(sq[:, :tcn, :], xw_c[:, :tcn, :],
                                        xw_c[:, :tcn, :], AL.mult)
                sqh = sq[:, :, : EMB // 2]
                nc.vector.tensor_tensor(
                    sqh[:, :tcn, :], sq[:, :tcn, : EMB // 2],
                    sq[:, :tcn, EMB // 2 :], AL.add)
                xh = sq[:, :, EMB // 2 :]
                nc.vector.tensor_tensor(
                    xh[:, :tcn, :], xw_c[:, :tcn, : EMB // 2],
                    xw_c[:, :tcn, EMB // 2 :], AL.add)
                sx = tpool.tile([128, CHUNK_TILES], dt.float32, tag="sxc")
                nc.vector.reduce_sum(sx[:, :tcn], xh[:, :tcn, :],
                                     axis=mybir.AxisListType.X)
                sxx = tpool.tile([128, CHUNK_TILES], dt.float32, tag="sxxc")
                nc.vector.reduce_sum(sxx[:, :tcn], sqh[:, :tcn, :],
                                     axis=mybir.AxisListType.X)
                inv = 1.0 / EMB
                mu = tpool.tile([128, CHUNK_TILES], dt.float32, tag="muc")
                nc.vector.tensor_scalar_mul(mu[:, :tcn], sx[:, :tcn], inv)
                veps = tpool.tile([128, CHUNK_TILES], dt.float32, tag="vepsc")
                nc.vector.tensor_scalar(veps[:, :tcn], sxx[:, :tcn], inv, EPS,
                                        AL.mult, AL.add)
                nmusq = tpool.tile([128, CHUNK_TILES], dt.float32, tag="nmusqc")
                nc.vector.scalar_tensor_tensor(
                    nmusq[:, :tcn], mu[:, :tcn], -1.0, mu[:, :tcn], AL.mult, AL.mult)
                nc.vector.tensor_tensor(veps[:, :tcn], veps[:, :tcn],
                                        nmusq[:, :tcn], AL.add)
                rstd_t = rsqrt_newton(nc, tpool, veps[:, :tcn], tcn, "crs")
                nmr_c = tpool.tile([128, CHUNK_TILES], dt.float32, tag="nmrc")
                nc.vector.scalar_tensor_tensor(
                    nmr_c[:, :tcn], mu[:, :tcn], -1.0, rstd_t, AL.mult, AL.mult)

                for ti in range(tcn):
                    blk, isfirst, islast = blk_of_tile[t0c + ti]
                    act = wpool.tile([128, EMB], dt.bfloat16, tag="act")
                    if fl_triv:
                        nc.scalar.activation(
                            act[:], xw_c[:, ti, :], LR,
                            bias=nmr_c[:, ti : ti + 1],
                            scale=rstd_t[:, ti : ti + 1], alpha=SLOPE)
                    else:
                        y1 = wpool.tile([128, EMB], dt.float32, tag="y1")
                        nc.vector.tensor_scalar(
                            y1[:], xw_c[:, ti, :], mu[:, ti : ti + 1],
                            rstd_t[:, ti : ti + 1], AL.subtract, AL.mult)
                        y2 = wpool.tile([128, EMB], dt.float32, tag="y2")
                        nc.vector.scalar_tensor_tensor(
                            y2[:], y1[:], 1.0, cw[pre + "flg"][:], AL.mult, AL.mult)
                        y3 = wpool.tile([128, EMB], dt.float32, tag="y3")
                        nc.vector.tensor_add(y3[:], y2[:], cw[pre + "flb"][:])
                        nc.scalar.activation(act[:], y3[:], LR,
                                             bias=zero_col[:], alpha=SLOPE)
                    if cur_ps[0] is None:
                        psb_new = psagg.tile([128, EMB], dt.float32, tag="agg")
                        cur_ps[0] = psb_new
                    psb = cur_ps[0]
                    nc.tensor.matmul(
                        psb[:], ohe[:, ti * 128 : (ti + 1) * 128], act[:],
                        start=isfirst, stop=islast)
                    if islast:
                        nc.vector.tensor_add(acc[:, blk, :], acc[:, blk, :], psb[:])
                        cur_ps[0] = None

            # =========== conv post ===========
            def conv_post(cv, pre, acc, rightT, outT, lpout_name, lpout_w, lpout_b,
                          n_valid, cnt_sb, eps_sb):
                nblocks = cv.nblocks
                for g0 in range(0, nblocks, 8):
                    gb = min(8, nblocks - g0)
                    ub = postp.tile([128, 8, EMB], dt.float32, tag="pub")
                    for k in range(gb):
                        blk = g0 + k
                        accT = wpool.tile([128, 128], dt.bfloat16, tag="accT")
                        transpose_to(accT[:], acc[:, blk, :], 128, 128)
                        ps = psA.tile([128, EMB], dt.float32, tag="ps")
                        nc.tensor.matmul(ps[:], accT[:], cw[pre + "wf"][:],
                                         start=True, stop=True)
                        nc.vector.scalar_tensor_tensor(
                            ub[:, k, :], cw[pre + "bf"][:], cnt_sb[:, blk : blk + 1],
                            ps[:], AL.mult, AL.add)
                    psx = tpool.tile([128, 8], dt.float32, tag="psx")
                    nc.vector.reduce_sum(psx[:, :gb], ub[:, :gb, :],
                                         axis=mybir.AxisListType.X)
                    sqg = postp.tile([128, 8, EMB], dt.float32, tag="psqg")
                    nc.vector.tensor_tensor(sqg[:, :gb, :], ub[:, :gb, :],
                                            ub[:, :gb, :], AL.mult)
                    psxx = tpool.tile([128, 8], dt.float32, tag="psxx")
                    nc.vector.reduce_sum(psxx[:, :gb], sqg[:, :gb, :],
                                         axis=mybir.AxisListType.X)
                    inv = 1.0 / EMB
                    pmu = tpool.tile([128, 8], dt.float32, tag="pmu")
                    nc.vector.tensor_scalar_mul(pmu[:, :gb], psx[:, :gb], inv)
                    pveps = tpool.tile([128, 8], dt.float32, tag="pveps")
                    nc.vector.scalar_tensor_tensor(
                        pveps[:, :gb], psxx[:, :gb], inv, eps_sb[:, g0 : g0 + gb],
                        AL.mult, AL.add)
                    pnmusq = tpool.tile([128, 8], dt.float32, tag="pnmusq")
                    nc.vector.scalar_tensor_tensor(
                        pnmusq[:, :gb], pmu[:, :gb], -1.0, pmu[:, :gb],
                        AL.mult, AL.mult)
                    nc.vector.tensor_tensor(pveps[:, :gb], pveps[:, :gb],
                                            pnmusq[:, :gb], AL.add)
                    prstd_t = rsqrt_newton(nc, tpool, pveps[:, :gb], gb, "prs")
                    for k in range(gb):
                        blk = g0 + k
                        lo = blk * 128
                        nv = min(128, n_valid - lo)
                        lnv = wpool.tile([128, EMB], dt.bfloat16, tag="lnv")
                        nc.vector.tensor_scalar(
                            lnv[:], ub[:, k, :], pmu[:, k : k + 1],
                            prstd_t[:, k : k + 1], AL.subtract, AL.mult)
                        lnT = wpool.tile([128, 128], dt.bfloat16, tag="lnT")
                        transpose_to(lnT[:], lnv[:], 128, 128)
                        ps2 = psA.tile([128, EMB], dt.float32, tag="ps")
                        nc.tensor.matmul(ps2[:], lnT[:], cw[pre + "wo1a"][:],
                                         start=True, stop=False)
                        nc.tensor.matmul(ps2[:], rightT[:, lo : lo + 128],
                                         cw[pre + "wo1b"][:], start=False, stop=True)
                        u2 = wpool.tile([128, EMB], dt.float32, tag="pcu2")
                        nc.vector.tensor_add(u2[:], ps2[:], cw[pre + "bo1"][:])
                        tml = wpool.tile([128, EMB], dt.bfloat16, tag="tml")
                        nc.scalar.activation(tml[:], u2[:], LR, bias=zero_col[:],
                                             alpha=SLOPE)
                        tT = wpool.tile([128, 128], dt.bfloat16, tag="tT")
                        transpose_to(tT[:], tml[:], 128, 128)
                        ps3 = psA.tile([128, EMB], dt.float32, tag="ps")
                        nc.tensor.matmul(ps3[:], tT[:], cw[pre + "wo2"][:],
                                         start=True, stop=True)
                        u3 = wpool.tile([128, EMB], dt.float32, tag="pcu3")
                        nc.vector.tensor_add(u3[:], ps3[:], cw[pre + "bo2"][:])
                        res = wpool.tile([128, EMB], dt.bfloat16, tag="res")
                        nc.scalar.copy(res[:], u3[:])
                        transpose_to(outT[:, lo : lo + 128], res[:], 128, 128)
                        if lpout_name is not None and nv > 0:
                            ps4 = psA.tile([128, EMB], dt.float32, tag="ps")
                            nc.tensor.matmul(ps4[:], outT[:, lo : lo + 128],
                                             cw[lpout_w][:], start=True, stop=True)
                            ub4 = wpool.tile([128, EMB], dt.float32, tag="pc4u")
                            nc.vector.tensor_add(ub4[:], ps4[:], cw[lpout_b][:])
                            ob = wpool.tile([128, EMB], dt.bfloat16, tag="pc4o")
                            nc.scalar.copy(ob[:], ub4[:])
                            nc.sync.dma_start(b.d[lpout_name][lo : lo + nv, :],
                                              ob[:nv, :])

            # ---- flattened chunk pipeline ----
            n1 = sum(1 for ch in chunks if ch["conv"] == 1)
            for k, ch in enumerate(chunks):
                if ch["conv"] == 2 and k == n1:
                    # conv1 done: post + AllGather lp2 (GPSIMD keeps prepping
                    # conv2 chunks already in the window)
                    conv_post(c1p, "vc_", acc1, c0T, c1T, "lp2_loc", "cv_wl",
                              "cv_bl", NCL, cnt1, eps1)
                    gp_chain(nc.gpsimd.collective_compute(
                        "AllGather", AL.bypass, ins=[lp2_loc[:]],
                        outs=[lp2_full[:]], replica_groups=[list(range(NCORES))]))
                emit_trigger(ch)
                if k + PREP_AHEAD < len(chunks):
                    emit_prep(chunks[k + PREP_AHEAD])
                process(ch)

            conv_post(c2p, "cv_", acc2, v0T, v1T, None, None, None,
                      NVL, cnt2, eps2)

            # =========== heads ===========
            if nact == 0:
                zrow = wpool.tile([1, 512], dt.bfloat16, tag="orow")
                nc.vector.memset(zrow[:], 0.0)
                for j in range(NVLh // 512):
                    nc.sync.dma_start(out_d[:, j * 512 : (j + 1) * 512], zrow[:])
            else:
                nch = NVLh // 512
                for j in range(nch):
                    pso = psout.tile([1, 512], dt.float32, tag="pso")
                    for hi in range(nact):
                        hw1t = hw1pool.tile([128, 128], dt.bfloat16, tag="hw1t")
                        nc.sync.dma_start(hw1t[:], b.d["hw1"][:, hi, :])
                        ps = psA.tile([128, 512], dt.float32, tag="ps")
                        nc.tensor.matmul(ps[:], hw1t[:],
                                         v1T[:, j * 512 : (j + 1) * 512],
                                         start=True, stop=True)
                        hh = wpool.tile([128, 512], dt.bfloat16, tag="hh")
                        if hi % 7 < 2:
                            zt = headp.tile([128, 512], dt.float32, tag="hzt")
                            nc.vector.tensor_scalar(
                                zt[:], ps[:], cw["hb1"][:, hi : hi + 1], None,
                                AL.add)
                            st2 = headp.tile([128, 512], dt.bfloat16, tag="hst")
                            nc.vector.tensor_scalar_mul(st2[:], zt[:], SLOPE)
                            nc.vector.tensor_tensor(hh[:], zt[:], st2[:], AL.max)
                        else:
                            nc.scalar.activation(hh[:], ps[:], LR,
                                                 bias=cw["hb1"][:, hi : hi + 1],
                                                 scale=1.0, alpha=SLOPE)
                        nc.tensor.matmul(pso[:], cw["hw2"][:, hi : hi + 1], hh[:],
                                         start=(hi == 0), stop=(hi == nact - 1))
                    orow = cpool.tile([1, 512], dt.bfloat16, tag="orow")
                    nc.scalar.copy(orow[:], pso[:])
                    nc.sync.dma_start(out_d[:, j * 512 : (j + 1) * 512], orow[:])

    nc.compile()
    return b


_CACHE = {}


def kernel(**inputs):
    key = tuple(sorted((k, tuple(np.asarray(v).shape)) for k, v in inputs.items()))
    p = host_prep(inputs)
    ck = (key, p["nact"], p["conv1"].etot, p["conv2"].etot,
          p["vc_fl_trivial"], p["cv_fl_trivial"])
    if ck in _CACHE:
        b = _CACHE[ck]
    else:
        b = build_program(p)
        _CACHE[ck] = b
    in_maps = [dict(p["core_inputs"][c]) for c in range(NCORES)]
    res = run_bass_kernel_spmd(b.nc, in_maps, core_ids=list(range(NCORES)))
    NVL = p["NVL"]
    out = np.concatenate([res.results[c]["out"][0, :NVL] for c in range(NCORES)])
    out = out.astype(np.float32) * p["out_scale"] + p["out_add"]
    return out.astype(np.float32)


# revision 14
# speedup vs baseline: 1.0333x; 1.0203x over previous
"""Trainium2 Bass kernel for BipartiteGCN (8 NeuronCores, SPMD). v2

Strategy (v2 — scheduling-first rewrite):
 - Node rows sharded 8 ways; edges sharded by DESTINATION range, sorted by
   dst block then src; per-edge lp[src] rows fetched via SWDGE dma_gather.
 - SWDGE descriptor generation (~7.7ns/row, ~2.0ms/core total) is the hard
   floor; everything else must overlap it:
     * gathers issued as prepare_only descriptor preps with a W-deep
       sliding window flattened across BOTH convs; triggers fire when the
       lp tables land. GPSIMD starts prepping at t=0 (during embeds) and
       never idles.
     * pools are deep enough that DVE stats of chunk k never WAR-wait on
       gather k-4 completions (the v1 failure mode: 1.7ms of DVE stalls).
 - Scatter-mean division eliminated exactly: LN is scale-invariant per row,
   so LN(agg/cnt + bf) == LN(agg + cnt*bf) with eps -> eps*max(cnt,1)^2
   (cnt is host-known index data, shipped as per-partition constants).
 - All rsqrt on DVE via Newton iteration (no ACT Sqrt table churn).
 - Embed / conv-post / heads as v1 otherwise.
"""

import os
import sys

for _p in ("/opt/trn_rl_repo",):
    if _p not in sys.path:
        sys.path.insert(0, _p)

import numpy as np
import ml_dtypes

import concourse.bass as bass
import concourse.bacc as bacc
import concourse.mybir as mybir
from concourse import tile, library_config
from concourse.bass_utils import run_bass_kernel_spmd
from concourse import hw_specs as _hw_specs

# Calibrated SWDGE dma_gather descriptor-generation rate (measured ~7.7
# ns/descriptor on HW; the stock 0.34 makes the Tile scheduler mis-order).
_hw_specs.TRN2Spec.SWDGE_NS_PER_DESCRIPTOR = 7.7

BF16 = ml_dtypes.bfloat16
F32 = np.float32
NCORES = 8
EMB = 128
CHUNK_TILES = 20     # tiles (128 edges) per dma_gather
PREP_AHEAD = 4       # gather chunks prepped ahead of their trigger
PREP_MODE = os.environ.get("KPREP", "prep")  # "prep" | "inline"
HI_BASE = 32768
EPS = 1e-5
SLOPE = 0.01

dt = mybir.dt


def _wrap_idx(idx_i16):
    """[N] int16 -> [128, N//16] wrapped (i at [i%16, i//16]) + replicated 8x."""
    n = idx_i16.shape[0]
    assert n % 16 == 0
    w = idx_i16.reshape(n // 16, 16).T
    return np.tile(w, (8, 1)).copy()


def _bcast_row(v, rows=128):
    return np.broadcast_to(np.asarray(v, F32)[None, :], (rows, v.shape[0])).copy()


class ConvPrep:
    """Per-conv edge-sharding data. Same segment layout for all cores."""

    def __init__(self, dst, src, n_dst, n_src, dst_per_core):
        self.n_dst_local = dst_per_core
        self.nblocks = -(-dst_per_core // 128)
        nb = self.nblocks
        self.two_buckets = n_src > HI_BASE
        nu = 2 if self.two_buckets else 1
        self.nu = nu

        core = dst // dst_per_core
        dloc_all = dst - core * dst_per_core
        block_all = dloc_all // 128

        per = [[[None] * nb for _ in range(nu)] for _ in range(NCORES)]
        for c in range(NCORES):
            m = core == c
            d_c = dloc_all[m]
            s_c = src[m]
            b_c = block_all[m]
            u_c = (s_c >= HI_BASE).astype(np.int8) if self.two_buckets else np.zeros(
                len(s_c), np.int8
            )
            for u in range(nu):
                mu = u_c == u
                db, sb, bb = d_c[mu], s_c[mu], b_c[mu]
                order = np.argsort(bb, kind="stable")
                db, sb, bb = db[order], sb[order], bb[order]
                bounds = np.searchsorted(bb, np.arange(nb + 1))
                for b in range(nb):
                    lo, hi = bounds[b], bounds[b + 1]
                    o2 = np.argsort(sb[lo:hi], kind="stable")
                    per[c][u][b] = (sb[lo:hi][o2], db[lo:hi][o2])

        self.ntiles = np.zeros((nu, nb), np.int64)
        for u in range(nu):
            for b in range(nb):
                mx = max(len(per[c][u][b][0]) for c in range(NCORES))
                self.ntiles[u, b] = -(-mx // 128) if mx > 0 else 0

        etot = int(self.ntiles.sum()) * 128
        self.etot = etot

        # host-known scatter-mean counts: cnt and EPS*max(cnt,1)^2
        self.cnt = np.zeros((NCORES, 128, nb), F32)
        self.eps2 = np.zeros((NCORES, 128, nb), F32)
        for c in range(NCORES):
            cnt = np.bincount(dloc_all[core == c], minlength=nb * 128).astype(F32)
            cnt = cnt[: nb * 128].reshape(nb, 128).T  # [128, nb]
            self.cnt[c] = cnt
            self.eps2[c] = EPS * np.maximum(cnt, 1.0) ** 2

        self.src_idx = np.zeros((NCORES, etot), np.int16)
        self.dstrel = np.full((NCORES, etot), -1.0, F32)
        off = 0
        self.seg_offsets = {}
        for u in range(nu):
            for b in range(nb):
                g = int(self.ntiles[u, b])
                if g == 0:
                    continue
                self.seg_offsets[(u, b)] = off
                for c in range(NCORES):
                    sb, db = per[c][u][b]
                    n = len(sb)
                    s_adj = sb - (HI_BASE if u == 1 else 0)
                    self.src_idx[c, off : off + n] = s_adj.astype(np.int16)
                    self.dstrel[c, off : off + n] = (db - 128 * b).astype(F32)
                off += g * 128
        assert off == etot

        self.streams = []
        for u in range(nu):
            blocks = [(b, int(self.ntiles[u, b])) for b in range(nb) if self.ntiles[u, b] > 0]
            start = self.seg_offsets[(u, blocks[0][0])] if blocks else 0
            nt = sum(g for _, g in blocks)
            self.streams.append({"u": u, "blocks": blocks, "start_edge": start, "ntiles": nt})

    def core_arrays(self, c):
        dr = self.dstrel[c]
        i = np.nonzero(dr >= 0)[0]
        lane = i % 128
        tb = (i // 128) * 128
        d = dr[i].astype(np.int64)
        oh = np.zeros((128, self.etot), ml_dtypes.float8_e4m3)
        oh[lane, tb + d] = 1.0
        ohT = np.zeros((128, self.etot), ml_dtypes.float8_e4m3)
        ohT[d, tb + lane] = 1.0
        return _wrap_idx(self.src_idx[c]), oh, ohT


def host_prep(inputs):
    p = {}
    cons_x = np.asarray(inputs["cons_x"], F32)
    var_x = np.asarray(inputs["var_x"], F32)
    edge_cons = np.asarray(inputs["edge_cons"]).astype(np.int64)
    edge_var = np.asarray(inputs["edge_var"]).astype(np.int64)
    head_mask = np.asarray(inputs["head_mask"]).astype(bool)

    NC, CF = cons_x.shape
    NV, VF = var_x.shape
    assert NC % NCORES == 0 and NV % NCORES == 0
    NCL, NVL = NC // NCORES, NV // NCORES
    p.update(NC=NC, NV=NV, CF=CF, VF=VF, NCL=NCL, NVL=NVL)

    p["conv1"] = ConvPrep(edge_cons, edge_var, NC, NV, NCL)
    p["conv2"] = ConvPrep(edge_var, edge_cons, NV, NC, NVL)

    w = {}

    def embed_w(prefix, g, b, w1, b1, w2, b2, feat):
        w1 = np.asarray(w1, F32)
        w1g = np.asarray(g, F32)[:, None] * w1
        aug = np.concatenate([w1g, np.zeros((1, w1.shape[1]), F32)], 0)
        w[prefix + "w1aug"] = aug.astype(BF16)
        w[prefix + "s1"] = _bcast_row(np.asarray(b, F32) @ w1 + np.asarray(b1, F32))
        w[prefix + "r1"] = _bcast_row(w1g.sum(0))
        w[prefix + "w2"] = np.asarray(w2, F32).astype(BF16)
        w[prefix + "b2"] = _bcast_row(np.asarray(b2, F32))

    embed_w("ce_", inputs["ce_ln_g"], inputs["ce_ln_b"], inputs["ce_w1"],
            inputs["ce_b1"], inputs["ce_w2"], inputs["ce_b2"], CF)
    embed_w("ve_", inputs["ve_ln_g"], inputs["ve_ln_b"], inputs["ve_w1"],
            inputs["ve_b1"], inputs["ve_w2"], inputs["ve_b2"], VF)

    for pre in ("vc_", "cv_"):
        wl = np.asarray(inputs[pre + "wl"], F32)
        w[pre + "wl"] = wl.astype(BF16)
        w[pre + "bl"] = _bcast_row(np.asarray(inputs[pre + "bl"], F32))
        w[pre + "wr"] = np.asarray(inputs[pre + "wr"], F32).astype(BF16)
        flg = np.asarray(inputs[pre + "flg"], F32)
        flb = np.asarray(inputs[pre + "flb"], F32)
        p[pre + "fl_trivial"] = bool(np.all(flg == 1.0) and np.all(flb == 0.0))
        w[pre + "flg"] = _bcast_row(flg)
        w[pre + "flb"] = _bcast_row(flb)
        w[pre + "wf"] = np.asarray(inputs[pre + "wf"], F32).astype(BF16)
        w[pre + "bf"] = _bcast_row(np.asarray(inputs[pre + "bf"], F32))
        wo1 = np.asarray(inputs[pre + "wo1"], F32)
        plg = np.asarray(inputs[pre + "plg"], F32)
        plb = np.asarray(inputs[pre + "plb"], F32)
        w[pre + "wo1a"] = (plg[:, None] * wo1[:EMB]).astype(BF16)
        w[pre + "wo1b"] = wo1[EMB:].astype(BF16)
        w[pre + "bo1"] = _bcast_row(np.asarray(inputs[pre + "bo1"], F32) + plb @ wo1[:EMB])
        w[pre + "wo2"] = np.asarray(inputs[pre + "wo2"], F32).astype(BF16)
        w[pre + "bo2"] = _bcast_row(np.asarray(inputs[pre + "bo2"], F32))

    active = np.nonzero(head_mask)[0]
    nact = int(len(active))
    p["nact"] = nact
    denom = max(float(head_mask.sum()), 1.0)
    hb2 = np.asarray(inputs["hb2"], F32)
    p["out_scale"] = 1.0 / denom
    p["out_add"] = float(hb2[active].sum() / denom)
    if nact > 0:
        hw1 = np.asarray(inputs["hw1"], F32)[active]
        w["hw1"] = hw1.transpose(1, 0, 2).astype(BF16).copy()
        w["hb1"] = np.asarray(inputs["hb1"], F32)[active].T.copy()
        w["hw2"] = np.asarray(inputs["hw2"], F32)[active].T.astype(BF16).copy()

    w["identity"] = np.eye(128, dtype=BF16)
    p["weights"] = w

    NCLp = -(-NCL // 128) * 128
    NVLp = -(-NVL // 128) * 128
    p.update(NCLp=NCLp, NVLp=NVLp)
    NVLh = -(-NVL // 512) * 512
    p["NVLh"] = NVLh

    core_inputs = []
    for c in range(NCORES):
        m = {}
        cx = cons_x[c * NCL : (c + 1) * NCL]
        vx = var_x[c * NVL : (c + 1) * NVL]
        cxp = np.zeros((NCLp, CF), F32)
        cxp[:NCL] = cx
        vxp = np.zeros((NVLp, VF), F32)
        vxp[:NVL] = vx
        m["cons_rows"] = cxp.reshape(NCLp // 128, 128, CF).transpose(1, 0, 2).copy()
        m["var_rows"] = vxp.reshape(NVLp // 128, 128, VF).transpose(1, 0, 2).copy()
        m["consT_aug"] = np.concatenate([cxp.T, np.ones((1, NCLp), F32)], 0).astype(BF16)
        m["varT_aug"] = np.concatenate([vxp.T, np.ones((1, NVLp), F32)], 0).astype(BF16)
        s1, oh1, ohT1 = p["conv1"].core_arrays(c)
        m["e1_src"], m["e1_oh"], m["e1_ohT"] = s1, oh1, ohT1
        s2, oh2, ohT2 = p["conv2"].core_arrays(c)
        m["e2_src"], m["e2_oh"], m["e2_ohT"] = s2, oh2, ohT2
        m["e1_cnt"] = p["conv1"].cnt[c]
        m["e1_eps"] = p["conv1"].eps2[c]
        m["e2_cnt"] = p["conv2"].cnt[c]
        m["e2_eps"] = p["conv2"].eps2[c]
        for k, v in w.items():
            m[k] = v
        core_inputs.append(m)
    p["core_inputs"] = core_inputs
    return p


# ---------------------------------------------------------------------------


class B:
    def __init__(self, p):
        self.p = p
        self.nc = bacc.Bacc("TRN2", target_bir_lowering=False, debug=False,
                            num_devices=NCORES, num_swdge_queues=2)
        self.d = {}

    def dram(self, name, shape, dtype, kind=None, addr_space=None):
        kw = {}
        if kind:
            kw["kind"] = kind
        if addr_space:
            kw["addr_space"] = addr_space
        t = self.nc.dram_tensor(name, list(shape), dtype, **kw)
        self.d[name] = t
        return t


MAGIC = 0x5F3759DF


def rsqrt_newton(nc, pool, src_ap, n, tag, iters=2):
    """1/sqrt(src) on DVE only. src_ap [128, n] f32 > 0."""
    AL = mybir.AluOpType
    sh = pool.tile([128, n], dt.int32, tag=tag + "sh")
    nc.vector.tensor_scalar(sh[:], src_ap.bitcast(dt.int32), 1, None,
                            AL.arith_shift_right)
    y0 = pool.tile([128, n], dt.int32, tag=tag + "y0")
    nc.vector.tensor_scalar(y0[:], sh[:], -1, MAGIC, AL.mult, AL.add)
    cur = y0[:].bitcast(dt.float32)
    h = pool.tile([128, n], dt.float32, tag=tag + "h")
    nc.vector.tensor_scalar_mul(h[:], src_ap, 0.5)
    for it in range(iters):
        yy = pool.tile([128, n], dt.float32, tag=tag + f"yy{it}")
        nc.vector.tensor_tensor(yy[:], cur, cur, AL.mult)
        nc.vector.tensor_tensor(yy[:], yy[:], h[:], AL.mult)
        nc.vector.tensor_scalar(yy[:], yy[:], -1.0, 1.5, AL.mult, AL.add)
        nxt = pool.tile([128, n], dt.float32, tag=tag + f"n{it}")
        nc.vector.tensor_tensor(nxt[:], cur, yy[:], AL.mult)
        cur = nxt[:]
    return cur


def build_program(p):
    b = B(p)
    nc = b.nc
    w = p["weights"]
    NCL, NVL, NCLp, NVLp = p["NCL"], p["NVL"], p["NCLp"], p["NVLp"]
    CF, VF = p["CF"], p["VF"]
    NC, NV = p["NC"], p["NV"]
    NVLh = p["NVLh"]
    nact = p["nact"]

    din = lambda n, s, t: b.dram(n, s, t, kind="ExternalInput")
    din("cons_rows", [128, NCLp // 128, CF], dt.float32)
    din("var_rows", [128, NVLp // 128, VF], dt.float32)
    din("consT_aug", [CF + 1, NCLp], dt.bfloat16)
    din("varT_aug", [VF + 1, NVLp], dt.bfloat16)
    c1p, c2p = p["conv1"], p["conv2"]
    din("e1_src", [128, c1p.etot // 16], dt.int16)
    din("e1_oh", [128, c1p.etot], dt.float8e4)
    din("e1_ohT", [128, c1p.etot], dt.float8e4)
    din("e2_src", [128, c2p.etot // 16], dt.int16)
    din("e2_oh", [128, c2p.etot], dt.float8e4)
    din("e2_ohT", [128, c2p.etot], dt.float8e4)
    din("e1_cnt", [128, c1p.nblocks], dt.float32)
    din("e1_eps", [128, c1p.nblocks], dt.float32)
    din("e2_cnt", [128, c2p.nblocks], dt.float32)
    din("e2_eps", [128, c2p.nblocks], dt.float32)
    for k, v in w.items():
        dtt = dt.bfloat16 if v.dtype == BF16 else (dt.int16 if v.dtype == np.int16 else dt.float32)
        din(k, list(v.shape), dtt)
    out_d = b.dram("out", [1, NVLh], dt.bfloat16, kind="ExternalOutput")

    lp1_loc = b.dram("lp1_loc", [NVL, EMB], dt.bfloat16)
    lp1_full = b.dram("lp1_full", [NV, EMB], dt.bfloat16, addr_space="Shared")
    rp1_loc = b.dram("rp1_loc", [NCL, EMB], dt.bfloat16)
    lp2_loc = b.dram("lp2_loc", [NCL, EMB], dt.bfloat16)
    lp2_full = b.dram("lp2_full", [NC, EMB], dt.bfloat16, addr_space="Shared")
    rp2_loc = b.dram("rp2_loc", [NVL, EMB], dt.bfloat16)

    LR = mybir.ActivationFunctionType.Lrelu
    AL = mybir.AluOpType

    with tile.TileContext(nc) as tc:
        nc.gpsimd.load_library(library_config.mlp)
        from contextlib import ExitStack
        with ExitStack() as _stack:
            _ep = _stack.enter_context
            cpool = _ep(tc.tile_pool(name="const", bufs=1))
            rpool = _ep(tc.tile_pool(name="resident", bufs=1))
            wpool = _ep(tc.tile_pool(name="work", bufs=3))
            tpool = _ep(tc.tile_pool(name="tiny", bufs=5))
            gpool = _ep(tc.tile_pool(name="gath", bufs=PREP_AHEAD + 1))
            sidxpool = _ep(tc.tile_pool(name="sidxp", bufs=PREP_AHEAD + 2))
            ohpool = _ep(tc.tile_pool(name="ohp", bufs=2))
            xwpool = _ep(tc.tile_pool(name="xwp", bufs=2))
            embp = _ep(tc.tile_pool(name="embp", bufs=1))
            sqpool = _ep(tc.tile_pool(name="sqp", bufs=2))
            hw1pool = _ep(tc.tile_pool(name="hw1p", bufs=8))
            postp = _ep(tc.tile_pool(name="postp", bufs=2))
            headp = _ep(tc.tile_pool(name="headp", bufs=2))
            psA = _ep(tc.tile_pool(name="psA", bufs=4, space="PSUM"))
            psT = _ep(tc.tile_pool(name="psT", bufs=1, space="PSUM"))
            psagg = _ep(tc.tile_pool(name="psagg", bufs=2, space="PSUM"))
            psout = _ep(tc.tile_pool(name="psout", bufs=1, space="PSUM"))
            # ---- constants ----
            cw = {}
            for k, v in w.items():
                if k == "hw1":
                    continue  # streamed during the head stage
                dtt = dt.bfloat16 if v.dtype == BF16 else dt.float32
                t = cpool.tile(list(v.shape), dtt, tag=k)
                nc.sync.dma_start(t[:], b.d[k][:])
                cw[k] = t

            ident = cw["identity"]
            zero_col = cpool.tile([128, 1], dt.float32, tag="zero_col")
            nc.vector.memset(zero_col[:], 0.0)

            cnt1 = cpool.tile([128, c1p.nblocks], dt.float32, tag="cnt1")
            nc.sync.dma_start(cnt1[:], b.d["e1_cnt"][:])
            eps1 = cpool.tile([128, c1p.nblocks], dt.float32, tag="eps1")
            nc.sync.dma_start(eps1[:], b.d["e1_eps"][:])
            cnt2 = cpool.tile([128, c2p.nblocks], dt.float32, tag="cnt2")
            nc.sync.dma_start(cnt2[:], b.d["e2_cnt"][:])
            eps2 = cpool.tile([128, c2p.nblocks], dt.float32, tag="eps2")
            nc.sync.dma_start(eps2[:], b.d["e2_eps"][:])

            # residents
            c0T = rpool.tile([128, NCLp], dt.bfloat16, tag="c0T")
            v0T = rpool.tile([128, NVLp], dt.bfloat16, tag="v0T")
            c1T = rpool.tile([128, NCLp], dt.bfloat16, tag="c1T")
            v1T = rpool.tile([128, NVLh], dt.bfloat16, tag="v1T")
            nc.vector.memset(v1T[:], 0.0)
            acc1 = rpool.tile([128, c1p.nblocks, EMB], dt.bfloat16, tag="acc1")
            nc.vector.memset(acc1[:], 0.0)
            acc2 = rpool.tile([128, c2p.nblocks, EMB], dt.bfloat16, tag="acc2")
            nc.vector.memset(acc2[:], 0.0)

            # ---- SWDGE prep/trigger machinery ----
            dma_sems = [nc.alloc_semaphore("gq0"), nc.alloc_semaphore("gq1")]
            prep_sem = nc.alloc_semaphore("prep_ctr")
            gp_last = [None]

            def gp_chain(inst):
                if gp_last[0] is not None:
                    deps = bass.InstructionNameOrderedSet()
                    deps.add(gp_last[0])
                    inst.ins.add_nosync_dependencies_from(deps)
                gp_last[0] = inst.ins.name
                return inst

            # flattened chunk list across conv1 then conv2
            chunks = []
            for conv_id, cv, lp_dram, src_d in (
                (1, c1p, lp1_full, b.d["e1_src"]),
                (2, c2p, lp2_full, b.d["e2_src"]),
            ):
                for stream in cv.streams:
                    base_edge = stream["start_edge"]
                    ntiles = stream["ntiles"]
                    view_lo = HI_BASE if stream["u"] == 1 else 0
                    blk_of_tile = {}
                    t0 = 0
                    for (blk, tcnt) in stream["blocks"]:
                        for t in range(t0, t0 + tcnt):
                            blk_of_tile[t] = (blk, t == t0, t == t0 + tcnt - 1)
                        t0 += tcnt
                    tdone = 0
                    while tdone < ntiles:
                        tcn = min(CHUNK_TILES, ntiles - tdone)
                        chunks.append(dict(
                            conv=conv_id, lp=lp_dram, src_d=src_d,
                            view_lo=view_lo, e0=base_edge + tdone * 128,
                            t0=tdone, tcn=tcn, blk_of_tile=blk_of_tile,
                        ))
                        tdone += tcn
            for gi, ch in enumerate(chunks):
                ch["q"] = gi % 2

            prep_count = [0]

            def emit_prep(ch):
                ne = ch["tcn"] * 128
                e0 = ch["e0"]
                sidx = sidxpool.tile([128, CHUNK_TILES * 8], dt.int16, tag="sidx")
                nc.sync.dma_start(sidx[:, : ne // 16],
                                  ch["src_d"][:, e0 // 16 : (e0 + ne) // 16])
                g = gpool.tile([128, CHUNK_TILES, EMB], dt.bfloat16, tag="sgat")
                ch["g"] = g
                ch["sidx"] = sidx
                if PREP_MODE != "prep":
                    return
                lp_view = ch["lp"][ch["view_lo"]:, :] if ch["view_lo"] else ch["lp"][:, :]
                inst = nc.gpsimd.dma_gather(
                    g[:, : ch["tcn"], :], lp_view, sidx[:, : ne // 16], ne, ne,
                    EMB, single_packet=False, prepare_only=True,
                    sem=dma_sems[ch["q"]], queue_num=ch["q"])
                inst.then_inc(prep_sem, 1)
                gp_chain(inst)
                prep_count[0] += 1
                ch["prep_no"] = prep_count[0]

            def emit_trigger(ch):
                if PREP_MODE != "prep":
                    ne = ch["tcn"] * 128
                    lp_view = (ch["lp"][ch["view_lo"]:, :] if ch["view_lo"]
                               else ch["lp"][:, :])
                    gp_chain(nc.gpsimd.dma_gather(
                        ch["g"][:, : ch["tcn"], :], lp_view,
                        ch["sidx"][:, : ne // 16], ne, ne, EMB,
                        single_packet=False, queue_num=ch["q"]))
                    return
                gp_chain(nc.gpsimd.wait_ge(prep_sem, ch["prep_no"]))
                gp_chain(nc.gpsimd.trigger_dma(count=1, queue_num=ch["q"]))

            # =========== embeddings ===========
            def transpose_to(dst_ap, src_ap, n_p, n_f):
                ps = psT.tile([128, 128], dt.bfloat16, tag="psT")
                nc.tensor.transpose(ps[:n_f, :n_p], src_ap, ident[:n_p, :n_p])
                nc.scalar.copy(dst_ap, ps[:n_f, :n_p])

            def embed(pre, xT_aug_name, rows_name, nrows_p, nfeat, outT, extra):
                nchunks = nrows_p // 128
                xall = embp.tile([128, nchunks, nfeat], dt.float32, tag="embx")
                nc.sync.dma_start(xall[:], b.d[rows_name][:])
                sx = tpool.tile([128, nchunks], dt.float32, tag="esx")
                nc.vector.reduce_sum(sx[:], xall[:], axis=mybir.AxisListType.X)
                nc.vector.tensor_tensor(xall[:], xall[:], xall[:], AL.mult)
                sxx = tpool.tile([128, nchunks], dt.float32, tag="esxx")
                nc.vector.reduce_sum(sxx[:], xall[:], axis=mybir.AxisListType.X)
                inv = 1.0 / nfeat
                mu_b = tpool.tile([128, nchunks], dt.float32, tag="emub")
                nc.vector.tensor_scalar_mul(mu_b[:], sx[:], inv)
                veps = tpool.tile([128, nchunks], dt.float32, tag="evep")
                nc.vector.tensor_scalar(veps[:], sxx[:], inv, EPS, AL.mult, AL.add)
                nmusq = tpool.tile([128, nchunks], dt.float32, tag="enmu")
                nc.vector.scalar_tensor_tensor(
                    nmusq[:], mu_b[:], -1.0, mu_b[:], AL.mult, AL.mult)
                nc.vector.tensor_tensor(veps[:], veps[:], nmusq[:], AL.add)
                rstd_b = rsqrt_newton(nc, tpool, veps[:], nchunks, "erst")
                nrstd_b = tpool.tile([128, nchunks], dt.float32, tag="enrs")
                nc.vector.tensor_scalar_mul(nrstd_b[:], rstd_b, -1.0)
                for chn in range(nchunks):
                    xTa = wpool.tile([nfeat + 1, 128], dt.bfloat16, tag="xTa")
                    nc.sync.dma_start(xTa[:], b.d[xT_aug_name][:, chn * 128 : (chn + 1) * 128])
                    ps = psA.tile([128, EMB], dt.float32, tag="ps")
                    nc.tensor.matmul(ps[:], xTa[:],
                                     cw[pre + "w1aug"][:], start=True, stop=True)
                    tmid = wpool.tile([128, EMB], dt.float32, tag="embmid")
                    nc.vector.scalar_tensor_tensor(
                        tmid[:], cw[pre + "r1"][:], mu_b[:, chn : chn + 1], ps[:],
                        AL.mult, AL.subtract)
                    tmid2 = wpool.tile([128, EMB], dt.float32, tag="tmid2")
                    nc.vector.scalar_tensor_tensor(
                        tmid2[:], tmid[:], nrstd_b[:, chn : chn + 1], cw[pre + "s1"][:],
                        AL.mult, AL.add)
                    z1 = wpool.tile([128, EMB], dt.bfloat16, tag="z1")
                    nc.scalar.activation(z1[:], tmid2[:], LR, bias=zero_col[:], alpha=SLOPE)
                    z1T = wpool.tile([128, 128], dt.bfloat16, tag="z1T")
                    transpose_to(z1T[:], z1[:], 128, 128)
                    ps2 = psA.tile([128, EMB], dt.float32, tag="ps")
                    nc.tensor.matmul(ps2[:], z1T[:], cw[pre + "w2"][:], start=True, stop=True)
                    u = wpool.tile([128, EMB], dt.float32, tag="embu")
                    nc.vector.tensor_add(u[:], ps2[:], cw[pre + "b2"][:])
                    z2 = wpool.tile([128, EMB], dt.bfloat16, tag="z2")
                    nc.scalar.activation(z2[:], u[:], LR, bias=zero_col[:], alpha=SLOPE)
                    transpose_to(outT[:, chn * 128 : (chn + 1) * 128], z2[:], 128, 128)
                    for (wname, bname, dout, n_valid, odt) in extra:
                        lo = chn * 128
                        nv = min(128, max(0, n_valid - lo))
                        if nv == 0:
                            continue
                        ps3 = psA.tile([128, EMB], dt.float32, tag="ps")
                        nc.tensor.matmul(ps3[:], outT[:, lo : lo + 128],
                                         cw[wname][:], start=True, stop=True)
                        ob = wpool.tile([128, EMB], odt, tag="projo")
                        if bname is not None:
                            ub = wpool.tile([128, EMB], dt.float32, tag="proju")
                            nc.vector.tensor_add(ub[:], ps3[:], cw[bname][:])
                            nc.scalar.copy(ob[:], ub[:])
                        else:
                            nc.scalar.copy(ob[:], ps3[:])
                        nc.sync.dma_start(b.d[dout][lo : lo + nv, :], ob[:nv, :])

            # Preps for the first window BEFORE embeds: GPSIMD starts at t=0.
            n_pre = min(PREP_AHEAD, len(chunks))
            for gi in range(n_pre):
                emit_prep(chunks[gi])

            embed("ve_", "varT_aug", "var_rows", NVLp, VF, v0T,
                  [("vc_wl", "vc_bl", "lp1_loc", NVL, dt.bfloat16),
                   ("cv_wr", None, "rp2_loc", NVL, dt.bfloat16)])
            gp_chain(nc.gpsimd.collective_compute(
                "AllGather", AL.bypass, ins=[lp1_loc[:]], outs=[lp1_full[:]],
                replica_groups=[list(range(NCORES))]))
            embed("ce_", "consT_aug", "cons_rows", NCLp, CF, c0T,
                  [("vc_wr", None, "rp1_loc", NCL, dt.bfloat16)])

            # =========== conv edge-chunk processing ===========
            def _segs_per_block(cv):
                spb = {}
                for u in range(cv.nu):
                    for bb in range(cv.nblocks):
                        if cv.ntiles[u, bb] > 0:
                            spb[bb] = spb.get(bb, 0) + 1
                return spb

            conv_state = {
                1: dict(pre="vc_", rp_dram=rp1_loc, acc=acc1, n_valid=NCL,
                        oh_d=b.d["e1_oh"], ohT_d=b.d["e1_ohT"], rp_tiles={},
                        cur_ps=[None], segs_done={},
                        segs_per_block=_segs_per_block(c1p),
                        blocks_done=set(), post_next=0),
                2: dict(pre="cv_", rp_dram=rp2_loc, acc=acc2, n_valid=NVL,
                        oh_d=b.d["e2_oh"], ohT_d=b.d["e2_ohT"], rp_tiles={},
                        cur_ps=[None], segs_done={},
                        segs_per_block=_segs_per_block(c2p),
                        blocks_done=set(), post_next=0),
            }

            def get_rp(st, blk):
                if blk in st["rp_tiles"]:
                    return st["rp_tiles"][blk]
                rp_sb = wpool.tile([128, EMB], dt.bfloat16, tag="rpblk")
                lo = blk * 128
                nv = min(128, st["n_valid"] - lo)
                if nv < 128:
                    nc.vector.memset(rp_sb[:], 0.0)
                nc.sync.dma_start(rp_sb[:nv, :], st["rp_dram"][lo : lo + nv, :])
                # wpool recycles rpblk buffers every 3 allocations: keep only
                # the 2 most recent cached handles valid
                st["rp_tiles"][blk] = rp_sb
                while len(st["rp_tiles"]) > 2:
                    st["rp_tiles"].pop(next(iter(st["rp_tiles"])))
                return rp_sb

            def process_front(ch):
                st = conv_state[ch["conv"]]
                tcn = ch["tcn"]
                t0c = ch["t0"]
                e0 = ch["e0"]
                ne = tcn * 128
                sbuf = ch["g"]
                blk_of_tile = ch["blk_of_tile"]

                ohe = ohpool.tile([128, CHUNK_TILES * 128], dt.float8e4, tag="ohe")
                nc.sync.dma_start(ohe[:, :ne], st["oh_d"][:, e0 : e0 + ne])
                ohT = ohpool.tile([128, CHUNK_TILES * 128], dt.float8e4, tag="ohT")
                nc.sync.dma_start(ohT[:, :ne], st["ohT_d"][:, e0 : e0 + ne])
                ch["ohe"] = ohe

                xw_c = xwpool.tile([128, CHUNK_TILES, EMB], dt.bfloat16, tag="xwc")
                ch["xw"] = xw_c

                gi = 0
                while gi < tcn:
                    gn = min(4, tcn - gi)
                    psg = psA.tile([128, 4, EMB], dt.float32, tag="ps")
                    for k in range(gn):
                        ti = gi + k
                        blk, _, _ = blk_of_tile[t0c + ti]
                        rp_sb = get_rp(st, blk)
                        nc.tensor.matmul(psg[:, k, :],
                                         ohT[:, ti * 128 : (ti + 1) * 128],
                                         rp_sb[:], start=True, stop=True)
                    nc.vector.tensor_tensor(
                        xw_c[:, gi : gi + gn, :], sbuf[:, gi : gi + gn, :],
                        psg[:, :gn, :], AL.add)
                    gi += gn

                # chunk-batched LN stats (sq scratch separate from the gather
                # buffer so the gather ring frees right after the add)
                sq = sqpool.tile([128, CHUNK_TILES, EMB], dt.bfloat16, tag="sqc")
                nc.vector.tensor_tensor(sq[:, :tcn, :], xw_c[:, :tcn, :],
                                        xw_c[:, :tcn, :], AL.mult)
                sqh = sq[:, :, : EMB // 2]
                nc.vector.tensor_tensor(
                    sqh[:, :tcn, :], sq[:, :tcn, : EMB // 2],
                    sq[:, :tcn, EMB // 2 :], AL.add)
                xh = sq[:, :, EMB // 2 :]
                nc.vector.tensor_tensor(
                    xh[:, :tcn, :], xw_c[:, :tcn, : EMB // 2],
                    xw_c[:, :tcn, EMB // 2 :], AL.add)
                sx = tpool.tile([128, CHUNK_TILES], dt.float32, tag="sxc")
                nc.vector.reduce_sum(sx[:, :tcn], xh[:, :tcn, :],
                                     axis=mybir.AxisListType.X)
                sxx = tpool.tile([128, CHUNK_TILES], dt.float32, tag="sxxc")
                nc.vector.reduce_sum(sxx[:, :tcn], sqh[:, :tcn, :],
                                     axis=mybir.AxisListType.X)
                inv = 1.0 / EMB
                mu = tpool.tile([128, CHUNK_TILES], dt.float32, tag="muc")
                nc.vector.tensor_scalar_mul(mu[:, :tcn], sx[:, :tcn], inv)
                veps = tpool.tile([128, CHUNK_TILES], dt.float32, tag="vepsc")
                nc.vector.tensor_scalar(veps[:, :tcn], sxx[:, :tcn], inv, EPS,
                                        AL.mult, AL.add)
                nmusq = tpool.tile([128, CHUNK_TILES], dt.float32, tag="nmusqc")
                nc.vector.scalar_tensor_tensor(
                    nmusq[:, :tcn], mu[:, :tcn], -1.0, mu[:, :tcn], AL.mult, AL.mult)
                nc.vector.tensor_tensor(veps[:, :tcn], veps[:, :tcn],
                                        nmusq[:, :tcn], AL.add)
                rstd_t = rsqrt_newton(nc, tpool, veps[:, :tcn], tcn, "crs")
                nmr_c = tpool.tile([128, CHUNK_TILES], dt.float32, tag="nmrc")
                nc.vector.scalar_tensor_tensor(
                    nmr_c[:, :tcn], mu[:, :tcn], -1.0, rstd_t, AL.mult, AL.mult)
                ch["mu"] = mu
                ch["rstd"] = rstd_t
                ch["nmr"] = nmr_c

            def process_back(ch):
                st = conv_state[ch["conv"]]
                pre = st["pre"]
                fl_triv = p[pre + "fl_trivial"]
                tcn = ch["tcn"]
                t0c = ch["t0"]
                blk_of_tile = ch["blk_of_tile"]
                acc = st["acc"]
                cur_ps = st["cur_ps"]
                xw_c = ch["xw"]
                ohe = ch["ohe"]
                mu, rstd_t, nmr_c = ch["mu"], ch["rstd"], ch["nmr"]
                done_blocks = []

                for ti in range(tcn):
                    blk, isfirst, islast = blk_of_tile[t0c + ti]
                    act = wpool.tile([128, EMB], dt.bfloat16, tag="act")
                    if fl_triv:
                        nc.scalar.activation(
                            act[:], xw_c[:, ti, :], LR,
                            bias=nmr_c[:, ti : ti + 1],
                            scale=rstd_t[:, ti : ti + 1], alpha=SLOPE)
                    else:
                        y1 = wpool.tile([128, EMB], dt.float32, tag="y1")
                        nc.vector.tensor_scalar(
                            y1[:], xw_c[:, ti, :], mu[:, ti : ti + 1],
                            rstd_t[:, ti : ti + 1], AL.subtract, AL.mult)
                        y2 = wpool.tile([128, EMB], dt.float32, tag="y2")
                        nc.vector.scalar_tensor_tensor(
                            y2[:], y1[:], 1.0, cw[pre + "flg"][:], AL.mult, AL.mult)
                        y3 = wpool.tile([128, EMB], dt.float32, tag="y3")
                        nc.vector.tensor_add(y3[:], y2[:], cw[pre + "flb"][:])
                        nc.scalar.activation(act[:], y3[:], LR,
                                             bias=zero_col[:], alpha=SLOPE)
                    if cur_ps[0] is None:
                        psb_new = psagg.tile([128, EMB], dt.float32, tag="agg")
                        cur_ps[0] = psb_new
                    psb = cur_ps[0]
                    nc.tensor.matmul(
                        psb[:], ohe[:, ti * 128 : (ti + 1) * 128], act[:],
                        start=isfirst, stop=islast)
                    if islast:
                        nc.vector.tensor_add(acc[:, blk, :], acc[:, blk, :], psb[:])
                        cur_ps[0] = None
                        st["segs_done"][blk] = st["segs_done"].get(blk, 0) + 1
                        if st["segs_done"][blk] == st["segs_per_block"][blk]:
                            done_blocks.append(blk)
                return done_blocks

            # =========== conv post (emitted per group of 8 blocks) ===========
            def conv_post_group(cfg, g0):
                cv, pre, acc, rightT, outT = (cfg["cv"], cfg["pre"], cfg["acc"],
                                              cfg["rightT"], cfg["outT"])
                lpout_name, lpout_w, lpout_b = cfg["lpout"]
                n_valid, cnt_sb, eps_sb = cfg["n_valid"], cfg["cnt"], cfg["eps"]
                gb = min(8, cv.nblocks - g0)
                ub = postp.tile([128, 8, EMB], dt.float32, tag="pub")
                for k in range(gb):
                    blk = g0 + k
                    accT = wpool.tile([128, 128], dt.bfloat16, tag="accT")
                    transpose_to(accT[:], acc[:, blk, :], 128, 128)
                    ps = psA.tile([128, EMB], dt.float32, tag="ps")
                    nc.tensor.matmul(ps[:], accT[:], cw[pre + "wf"][:],
                                     start=True, stop=True)
                    nc.vector.scalar_tensor_tensor(
                        ub[:, k, :], cw[pre + "bf"][:], cnt_sb[:, blk : blk + 1],
                        ps[:], AL.mult, AL.add)
                psx = tpool.tile([128, 8], dt.float32, tag="psx")
                nc.vector.reduce_sum(psx[:, :gb], ub[:, :gb, :],
                                     axis=mybir.AxisListType.X)
                sqg = postp.tile([128, 8, EMB], dt.float32, tag="psqg")
                nc.vector.tensor_tensor(sqg[:, :gb, :], ub[:, :gb, :],
                                        ub[:, :gb, :], AL.mult)
                psxx = tpool.tile([128, 8], dt.float32, tag="psxx")
                nc.vector.reduce_sum(psxx[:, :gb], sqg[:, :gb, :],
                                     axis=mybir.AxisListType.X)
                inv = 1.0 / EMB
                pmu = tpool.tile([128, 8], dt.float32, tag="pmu")
                nc.vector.tensor_scalar_mul(pmu[:, :gb], psx[:, :gb], inv)
                pveps = tpool.tile([128, 8], dt.float32, tag="pveps")
                nc.vector.scalar_tensor_tensor(
                    pveps[:, :gb], psxx[:, :gb], inv, eps_sb[:, g0 : g0 + gb],
                    AL.mult, AL.add)
                pnmusq = tpool.tile([128, 8], dt.float32, tag="pnmusq")
                nc.vector.scalar_tensor_tensor(
                    pnmusq[:, :gb], pmu[:, :gb], -1.0, pmu[:, :gb],
                    AL.mult, AL.mult)
                nc.vector.tensor_tensor(pveps[:, :gb], pveps[:, :gb],
                                        pnmusq[:, :gb], AL.add)
                prstd_t = rsqrt_newton(nc, tpool, pveps[:, :gb], gb, "prs")
                for k in range(gb):
                    blk = g0 + k
                    lo = blk * 128
                    nv = min(128, n_valid - lo)
                    lnv = wpool.tile([128, EMB], dt.bfloat16, tag="lnv")
                    nc.vector.tensor_scalar(
                        lnv[:], ub[:, k, :], pmu[:, k : k + 1],
                        prstd_t[:, k : k + 1], AL.subtract, AL.mult)
                    lnT = wpool.tile([128, 128], dt.bfloat16, tag="lnT")
                    transpose_to(lnT[:], lnv[:], 128, 128)
                    ps2 = psA.tile([128, EMB], dt.float32, tag="ps")
                    nc.tensor.matmul(ps2[:], lnT[:], cw[pre + "wo1a"][:],
                                     start=True, stop=False)
                    nc.tensor.matmul(ps2[:], rightT[:, lo : lo + 128],
                                     cw[pre + "wo1b"][:], start=False, stop=True)
                    u2 = wpool.tile([128, EMB], dt.float32, tag="pcu2")
                    nc.vector.tensor_add(u2[:], ps2[:], cw[pre + "bo1"][:])
                    tml = wpool.tile([128, EMB], dt.bfloat16, tag="tml")
                    nc.scalar.activation(tml[:], u2[:], LR, bias=zero_col[:],
                                         alpha=SLOPE)
                    tT = wpool.tile([128, 128], dt.bfloat16, tag="tT")
                    transpose_to(tT[:], tml[:], 128, 128)
                    ps3 = psA.tile([128, EMB], dt.float32, tag="ps")
                    nc.tensor.matmul(ps3[:], tT[:], cw[pre + "wo2"][:],
                                     start=True, stop=True)
                    u3 = wpool.tile([128, EMB], dt.float32, tag="pcu3")
                    nc.vector.tensor_add(u3[:], ps3[:], cw[pre + "bo2"][:])
                    res = wpool.tile([128, EMB], dt.bfloat16, tag="res")
                    nc.scalar.copy(res[:], u3[:])
                    transpose_to(outT[:, lo : lo + 128], res[:], 128, 128)
                    if lpout_name is not None and nv > 0:
                        ps4 = psA.tile([128, EMB], dt.float32, tag="ps")
                        nc.tensor.matmul(ps4[:], outT[:, lo : lo + 128],
                                         cw[lpout_w][:], start=True, stop=True)
                        ub4 = wpool.tile([128, EMB], dt.float32, tag="pc4u")
                        nc.vector.tensor_add(ub4[:], ps4[:], cw[lpout_b][:])
                        ob = wpool.tile([128, EMB], dt.bfloat16, tag="pc4o")
                        nc.scalar.copy(ob[:], ub4[:])
                        nc.sync.dma_start(b.d[lpout_name][lo : lo + nv, :],
                                          ob[:nv, :])

            post_cfg = {
                1: dict(cv=c1p, pre="vc_", acc=acc1, rightT=c0T, outT=c1T,
                        lpout=("lp2_loc", "cv_wl", "cv_bl"), n_valid=NCL,
                        cnt=cnt1, eps=eps1),
                2: dict(cv=c2p, pre="cv_", acc=acc2, rightT=v0T, outT=v1T,
                        lpout=(None, None, None), n_valid=NVL,
                        cnt=cnt2, eps=eps2),
            }

            heads_emitted = [0]

            def head_chunk(j):
                pso = psout.tile([1, 512], dt.float32, tag="pso")
                for hi in range(nact):
                    hw1t = hw1pool.tile([128, 128], dt.bfloat16, tag="hw1t")
                    nc.sync.dma_start(hw1t[:], b.d["hw1"][:, hi, :])
                    ps = psA.tile([128, 512], dt.float32, tag="ps")
                    nc.tensor.matmul(ps[:], hw1t[:],
                                     v1T[:, j * 512 : (j + 1) * 512],
                                     start=True, stop=True)
                    hh = wpool.tile([128, 512], dt.bfloat16, tag="hh")
                    if hi % 7 < 2:
                        zt = headp.tile([128, 512], dt.float32, tag="hzt")
                        nc.vector.tensor_scalar(
                            zt[:], ps[:], cw["hb1"][:, hi : hi + 1], None,
                            AL.add)
                        st2 = headp.tile([128, 512], dt.bfloat16, tag="hst")
                        nc.vector.tensor_scalar_mul(st2[:], zt[:], SLOPE)
                        nc.vector.tensor_tensor(hh[:], zt[:], st2[:], AL.max)
                    else:
                        nc.scalar.activation(hh[:], ps[:], LR,
                                             bias=cw["hb1"][:, hi : hi + 1],
                                             scale=1.0, alpha=SLOPE)
                    nc.tensor.matmul(pso[:], cw["hw2"][:, hi : hi + 1], hh[:],
                                     start=(hi == 0), stop=(hi == nact - 1))
                orow = cpool.tile([1, 512], dt.bfloat16, tag="orow")
                nc.scalar.copy(orow[:], pso[:])
                nc.sync.dma_start(out_d[:, j * 512 : (j + 1) * 512], orow[:])

            def emit_ready_followups(conv_id):
                """Emit post groups whose blocks completed; for conv2, also
                head chunks whose v1T columns are final."""
                st = conv_state[conv_id]
                cfg = post_cfg[conv_id]
                cv = cfg["cv"]
                while st["post_next"] < cv.nblocks:
                    g0 = st["post_next"]
                    gb = min(8, cv.nblocks - g0)
                    if not all((g0 + i) in st["blocks_done"] for i in range(gb)):
                        break
                    conv_post_group(cfg, g0)
                    st["post_next"] = g0 + gb
                if conv_id == 2 and nact > 0:
                    nch = NVLh // 512
                    while heads_emitted[0] < nch:
                        j = heads_emitted[0]
                        hi_blk = min(4 * j + 3, c2p.nblocks - 1)
                        if hi_blk >= st["post_next"]:
                            break
                        head_chunk(j)
                        heads_emitted[0] += 1

            def back_and_followups(ch):
                done = process_back(ch)
                st = conv_state[ch["conv"]]
                st["blocks_done"].update(done)
                emit_ready_followups(ch["conv"])

            # ---- flattened chunk pipeline (1-chunk front/back skew) ----
            n1 = sum(1 for ch in chunks if ch["conv"] == 1)
            pending = None
            for k, ch in enumerate(chunks):
                if ch["conv"] == 2 and k == n1:
                    if pending is not None:
                        back_and_followups(pending)
                        pending = None
                    gp_chain(nc.gpsimd.collective_compute(
                        "AllGather", AL.bypass, ins=[lp2_loc[:]],
                        outs=[lp2_full[:]], replica_groups=[list(range(NCORES))]))
                emit_trigger(ch)
                if k + PREP_AHEAD < len(chunks):
                    emit_prep(chunks[k + PREP_AHEAD])
                process_front(ch)
                if pending is not None:
                    back_and_followups(pending)
                pending = ch
            if pending is not None:
                back_and_followups(pending)

            if nact == 0:
                zrow = wpool.tile([1, 512], dt.bfloat16, tag="orow")
                nc.vector.memset(zrow[:], 0.0)
                for j in range(NVLh // 512):
                    nc.sync.dma_start(out_d[:, j * 512 : (j + 1) * 512], zrow[:])

    nc.compile()
    return b


_CACHE = {}


def kernel(**inputs):
    key = tuple(sorted((k, tuple(np.asarray(v).shape)) for k, v in inputs.items()))
    p = host_prep(inputs)
    ck = (key, p["nact"], p["conv1"].etot, p["conv2"].etot,
          p["vc_fl_trivial"], p["cv_fl_trivial"])
    if ck in _CACHE:
        b = _CACHE[ck]
    else:
        b = build_program(p)
        _CACHE[ck] = b
    in_maps = [dict(p["core_inputs"][c]) for c in range(NCORES)]
    res = run_bass_kernel_spmd(b.nc, in_maps, core_ids=list(range(NCORES)))
    NVL = p["NVL"]
    out = np.concatenate([res.results[c]["out"][0, :NVL] for c in range(NCORES)])
    out = out.astype(np.float32) * p["out_scale"] + p["out_add"]
    return out.astype(np.float32)
